# revision 6
# baseline (speedup 1.0000x reference)
"""Trainium2 Bass kernel for the LRU LM (nn_LruLM), v3.

Sharding: each core takes chunk k of BOTH batches (8 chunks of 256 per batch).
The two batches are software-pipelined out of phase so each batch's per-layer
boundary-state AllGather + DVE scan window is covered by the other batch's
matmuls (in_proj / deferred MLP). Weights stream as bf16 (stationary), LN
stats run on f32r moving operands (1 cycle/row), the LRU scan stays f32 with
bf16 rotation tables and bf16 scan outputs. The complex scan is decomposed
into 2 real first-order scans (tensor_tensor_scan); cross-chunk states are
corrected with an 8-way per-layer per-batch AllGather of local end-states.
Logits are vocab-sharded (6284/core) from a bf16 AllGather of final
activations, bf16 weights, bf16 output (f32 + pb on host).
"""

import contextlib

import numpy as np
import ml_dtypes

import concourse.bacc as bacc
import concourse.mybir as mybir
import concourse.tile as tile
from concourse.bass_utils import run_bass_kernel_spmd

AF = mybir.ActivationFunctionType
OP = mybir.AluOpType
F32 = mybir.dt.float32
F32R = mybir.dt.float32r
BF16 = mybir.dt.bfloat16

V, D, L, B, S = 50257, 768, 6, 2, 2048
TC = 256                     # tokens per chunk per batch
T = 2 * TC                   # tokens per core (b0 cols | b1 cols)
NC = 8
NCH = 8                      # chunks per batch
CT = D // 128                # 6 channel tiles
VSH = 6284                   # vocab shard width (12*512 + 140)
VW = [512] * 12 + [140]
VOFF = [sum(VW[:i]) for i in range(13)]
EPS = 1e-5
ALL8 = [list(range(NC))]
# in_proj column order: (vr_i, vi_i) pairs first so each pair's rotation +
# scan starts as soon as its two psums land; o tiles afterwards.
PERM = [0, 6, 1, 7, 2, 8, 3, 9, 4, 10, 5, 11] + list(range(12, 24))


def _build(nc):
    d = {}
    d["x0t"] = nc.dram_tensor("x0t", [D, T], F32R, kind="ExternalInput")
    d["postc"] = nc.dram_tensor("postc", [L, D, TC], BF16, kind="ExternalInput")
    d["posts"] = nc.dram_tensor("posts", [L, D, TC], BF16, kind="ExternalInput")
    d["iotat"] = nc.dram_tensor("iotat", [128, TC], F32, kind="ExternalInput")
    d["cw"] = nc.dram_tensor("cw", [L, CT, 128, NCH], F32, kind="ExternalInput")
    for nm in ["nuv", "lnnu", "ln1g", "ln1b", "ln2g", "ln2b", "outbv", "b2v"]:
        d[nm] = nc.dram_tensor(nm, [128, CT * L], F32, kind="ExternalInput")
    for nm in ["lnrg", "lnrb"]:
        d[nm] = nc.dram_tensor(nm, [128, 2 * CT * L], F32, kind="ExternalInput")
    for nm in ["inbv", "b1v"]:
        d[nm] = nc.dram_tensor(nm, [128, 24 * L], F32, kind="ExternalInput")
    for nm in ["lnfg", "lnfb"]:
        d[nm] = nc.dram_tensor(nm, [128, CT], F32, kind="ExternalInput")
    d["w_in"] = nc.dram_tensor("w_in", [L, CT, 128, 24 * 128], BF16, kind="ExternalInput")
    d["w_out"] = nc.dram_tensor("w_out", [L, 2 * CT, 128, CT * 128], BF16, kind="ExternalInput")
    d["w_1"] = nc.dram_tensor("w_1", [L, CT, 128, 24 * 128], BF16, kind="ExternalInput")
    d["w_2"] = nc.dram_tensor("w_2", [L, 24, 128, CT * 128], BF16, kind="ExternalInput")
    d["pwt"] = nc.dram_tensor("pwt", [CT, 128, VSH], BF16, kind="ExternalInput")
    outp = nc.dram_tensor("outp", [NC * T, VSH], BF16, kind="ExternalOutput")

    cc_in = [[nc.dram_tensor(f"ccin{l}_{b}", [128, 2 * CT], F32) for b in range(2)]
             for l in range(L)]
    cc_out = [[nc.dram_tensor(f"ccout{l}_{b}", [NCH * 128, 2 * CT], F32)
               for b in range(2)] for l in range(L)]
    xf_in = [nc.dram_tensor(f"xfin{b}", [D, TC], BF16) for b in range(2)]
    xf_all = [nc.dram_tensor(f"xfall{b}", [NC * D, TC], BF16, addr_space="Shared")
              for b in range(2)]

    BS = (slice(0, TC), slice(TC, T))

    with tile.TileContext(nc) as tc:
        est = contextlib.ExitStack()
        with est:
            vec = est.enter_context(tc.tile_pool(name="vec", bufs=1))
            rowp = est.enter_context(tc.tile_pool(name="rowp", bufs=6))
            tmp3 = est.enter_context(tc.tile_pool(name="tmp3", bufs=4))
            tmp4 = est.enter_context(tc.tile_pool(name="tmp4", bufs=4))
            ps_sm = est.enter_context(tc.tile_pool(name="pssm", bufs=1, space="PSUM"))
            ps_bc = est.enter_context(tc.tile_pool(name="psbc", bufs=1, space="PSUM"))
            ps_mm = est.enter_context(tc.tile_pool(name="psmm", bufs=6, space="PSUM"))

            ones128f = vec.tile([128, 1], F32, tag="ones128f")
            nc.vector.memset(ones128f[:], 1.0)
            ones128r = vec.tile([128, 1], F32R, tag="ones128r")
            nc.vector.tensor_copy(ones128r[:], ones128f[:])
            onesrow_f = vec.tile([1, 128], F32, tag="onesrowf")
            nc.vector.memset(onesrow_f[:], 1.0)
            onesrow_r = vec.tile([1, 128], F32R, tag="onesrowr")
            nc.vector.tensor_copy(onesrow_r[:], onesrow_f[:])
            epst = vec.tile([1, 1], F32, tag="epst")
            nc.vector.memset(epst[:], EPS)
            iota_t = vec.tile([128, TC], F32, tag="iota")
            nc.sync.dma_start(iota_t[:], d["iotat"][:])
            cwt = vec.tile([128, L * CT * NCH], F32, tag="cwt")
            nc.sync.dma_start(
                cwt[:].rearrange("p (l c j) -> p l c j", l=L, c=CT),
                d["cw"][:].rearrange("l c p j -> p l c j"),
            )

            vt = {}
            for nm in ["nuv", "lnnu", "ln1g", "ln1b", "ln2g", "ln2b", "outbv",
                       "b2v", "lnrg", "lnrb", "inbv", "b1v", "lnfg", "lnfb"]:
                vt[nm] = vec.tile(list(d[nm].shape), F32, tag=nm, name=nm)
                nc.sync.dma_start(vt[nm][:], d[nm][:])

            def layer_norm(xaps, g_ap, b_ap, out_pool, out_dtype, out_tag):
                """LN over channels (partitions, across len(xaps) [128,TC] APs).
                Stats via f32r/bf16 matmuls; returns per-tile rhs tiles."""
                n = len(xaps)
                nch = float(n * 128)
                ps_st = ps_sm.tile([1, 2 * TC], F32, tag="st")
                for i in range(n):
                    nc.tensor.matmul(ps_st[:, 0:TC], ones128r[:], xaps[i],
                                     start=(i == 0), stop=(i == n - 1))
                for i in range(n):
                    sq = tmp3.tile([128, TC], F32R, tag="sqt", bufs=3)
                    nc.scalar.activation(sq[:], xaps[i], AF.Square)
                    nc.tensor.matmul(ps_st[:, TC:2 * TC], ones128r[:], sq[:],
                                     start=(i == 0), stop=(i == n - 1))
                m = rowp.tile([1, TC], F32, tag="lnrow")
                nc.vector.tensor_scalar_mul(m[:], ps_st[:, 0:TC], 1.0 / nch)
                mq = rowp.tile([1, TC], F32, tag="lnrow")
                nc.vector.tensor_scalar_mul(mq[:], ps_st[:, TC:2 * TC], 1.0 / nch)
                nm2 = rowp.tile([1, TC], F32, tag="lnrow")
                nc.vector.scalar_tensor_tensor(nm2[:], m[:], -1.0, m[:],
                                               OP.mult, OP.mult)
                var = rowp.tile([1, TC], F32, tag="lnrow")
                nc.vector.tensor_tensor(var[:], mq[:], nm2[:], OP.add)
                sd = rowp.tile([1, TC], F32, tag="lnrow")
                nc.scalar.activation(sd[:], var[:], AF.Sqrt, bias=epst[:])
                rstd = rowp.tile([1, TC], F32R, tag="lnrowr")
                with nc.allow_low_precision(reason="f32r bits are f32"):
                    nc.vector.reciprocal(rstd[:], sd[:])
                a0 = rowp.tile([1, TC], F32R, tag="lnrowr")
                nc.vector.scalar_tensor_tensor(a0[:], m[:], -1.0, rstd[:],
                                               OP.mult, OP.mult)
                bc = ps_bc.tile([128, 2 * TC], F32, tag="bc")
                nc.tensor.matmul(bc[:, 0:TC], onesrow_r[:], rstd[:],
                                 start=True, stop=True)
                nc.tensor.matmul(bc[:, TC:2 * TC], onesrow_r[:], a0[:],
                                 start=True, stop=True)
                outs = []
                for i in range(n):
                    t1 = tmp3.tile([128, TC], F32, tag="lnt1", bufs=4)
                    nc.vector.tensor_tensor(t1[:], xaps[i], bc[:, 0:TC], OP.mult)
                    t2 = tmp3.tile([128, TC], F32, tag="lnt2", bufs=4)
                    nc.vector.tensor_tensor(t2[:], t1[:], bc[:, TC:2 * TC], OP.add)
                    o = out_pool.tile([128, TC], out_dtype, tag=out_tag)
                    nc.vector.tensor_scalar(o[:], t2[:], g_ap(i), b_ap(i),
                                            OP.mult, OP.add)
                    outs.append(o)
                return outs

            def proj(n_k, n_mt, get_w, rhs_list, grp, group_cb):
                """psum[mi] = sum_k get_w(k, m0+mi).T @ rhs_list[k]."""
                for m0 in range(0, n_mt, grp):
                    g = min(grp, n_mt - m0)
                    psl = [ps_mm.tile([128, TC], F32, tag="mmps", name="mmps")
                           for _ in range(g)]
                    for ki in range(n_k):
                        for mi in range(g):
                            nc.tensor.matmul(psl[mi][:], get_w(ki, m0 + mi),
                                             rhs_list[ki][:],
                                             start=(ki == 0), stop=(ki == n_k - 1))
                    group_cb(m0, psl)

            tst = contextlib.ExitStack()
            with tst:
                xres = tst.enter_context(tc.tile_pool(name="xres", bufs=12))
                rhs = tst.enter_context(tc.tile_pool(name="rhs", bufs=16))
                tabs = tst.enter_context(tc.tile_pool(name="tabs", bufs=3))
                w3k = tst.enter_context(tc.tile_pool(name="w3k", bufs=12))
                w15 = tst.enter_context(tc.tile_pool(name="w15", bufs=14))
                abp = tst.enter_context(tc.tile_pool(name="abp", bufs=10))
                ABp = tst.enter_context(tc.tile_pool(name="ABp", bufs=13))
                sop = tst.enter_context(tc.tile_pool(name="sop", bufs=25))
                ggp = tst.enter_context(tc.tile_pool(name="ggp", bufs=12))
                ypp = tst.enter_context(tc.tile_pool(name="ypp", bufs=12))
                mpp = tst.enter_context(tc.tile_pool(name="mpp", bufs=6))
                scl = tst.enter_context(tc.tile_pool(name="scl", bufs=2))

                x = []
                for i in range(CT):
                    xt = xres.tile([128, T], F32R, tag="x")
                    nc.sync.dma_start(xt[:], d["x0t"][i * 128:(i + 1) * 128, :])
                    x.append(xt)

                pending_mlp = [None]  # deferred b1 MLP of previous layer

                def load_w15(dram, l, kts, c0, c1):
                    tiles = []
                    for kt in kts:
                        wt = w15.tile([128, c1 - c0], BF16, tag="w15", name="w15t")
                        nc.sync.dma_start(wt[:], dram[l, kt, :, c0:c1])
                        tiles.append(wt)
                    return tiles

                def do_mlp(l, b, x_in, ln2rhs, xtgt):
                    """w1 (2 quarter-calls per half) + w2 per hidden half.
                    Writes x_in + mlp in place into xtgt (b's half)."""
                    co = l * CT
                    bsl = BS[b]
                    mparts = {}
                    for p in range(2):
                        gl = [None] * 12
                        for qh in range(2):
                            q = 2 * p + qh
                            w1q = load_w15(d["w_1"], l, range(CT),
                                           q * 768, (q + 1) * 768)

                            def w1_cb(m0, psl, q=q, gl=gl):
                                for mi, ps in enumerate(psl):
                                    mt_abs = q * 6 + m0 + mi
                                    g = ggp.tile([128, TC], BF16, tag="gelu")
                                    nc.scalar.activation(
                                        g[:], ps[:], AF.Gelu,
                                        bias=vt["b1v"][:, l * 24 + mt_abs:
                                                       l * 24 + mt_abs + 1])
                                    gl[qh * 6 + m0 + mi] = g

                            proj(CT, 6,
                                 lambda ki, mt, w1q=w1q: w1q[ki][:, mt * 128:(mt + 1) * 128],
                                 ln2rhs, 2, w1_cb)
                        w2p = load_w15(d["w_2"], l, range(p * 12, p * 12 + 12),
                                       0, CT * 128)

                        def w2_cb(m0, psl, p=p):
                            for mi, ps in enumerate(psl):
                                mt = m0 + mi
                                if p == 0:
                                    pt = mpp.tile([128, TC], BF16, tag="mpart",
                                                  name="mpart")
                                    nc.scalar.activation(pt[:], ps[:], AF.Copy)
                                    mparts[mt] = pt
                                else:
                                    t = tmp3.tile([128, TC], F32, tag="m2t",
                                                  bufs=3, name="m2t")
                                    nc.vector.scalar_tensor_tensor(
                                        t[:], ps[:],
                                        vt["b2v"][:, co + mt:co + mt + 1],
                                        mparts[mt][:], OP.add, OP.add)
                                    nc.vector.tensor_tensor(
                                        xtgt[mt][:, bsl], t[:],
                                        x_in[mt][:, bsl], OP.add)

                        proj(12, 6,
                             lambda ki, mt, w2p=w2p: w2p[ki][:, mt * 128:(mt + 1) * 128],
                             gl, 2, w2_cb)

                for l in range(L):
                    co = l * CT
                    co2 = l * 2 * CT
                    pct = tabs.tile([128, CT * TC], BF16, tag="tab", name="pct")
                    nc.sync.dma_start(
                        pct[:].rearrange("p (c t) -> p c t", c=CT),
                        d["postc"][l].rearrange("(c p) t -> p c t", p=128),
                    )
                    psnt = tabs.tile([128, CT * TC], BF16, tag="tab", name="psnt")
                    nc.sync.dma_start(
                        psnt[:].rearrange("p (c t) -> p c t", c=CT),
                        d["posts"][l].rearrange("(c p) t -> p c t", p=128),
                    )

                    nubs = [None] * CT
                    npws = [None] * CT
                    AB = {}
                    so = {}
                    endAB = {}
                    inis = {}
                    xn = [None] * CT
                    ln2rhs = [None, None]

                    def pc_i(i):
                        return pct[:, i * TC:(i + 1) * TC]

                    def psn_i(i):
                        return psnt[:, i * TC:(i + 1) * TC]

                    def rot_scan(i, b, pvr, pvi, l=l, AB=AB, endAB=endAB,
                                 nubs=nubs):
                        if nubs[i] is None:
                            nub = abp.tile([128, TC], F32, tag="nub", bufs=6,
                                           name="nub")
                            nc.vector.tensor_scalar(
                                nub[:], iota_t[:], 0.0,
                                vt["nuv"][:, co + i:co + i + 1], OP.mult, OP.add)
                            nubs[i] = nub
                        bvr = vt["inbv"][:, l * 24 + i:l * 24 + i + 1]
                        bvi = vt["inbv"][:, l * 24 + CT + i:l * 24 + CT + i + 1]
                        t1 = abp.tile([128, TC], F32, tag="rt", bufs=4, name="rt1")
                        nc.vector.scalar_tensor_tensor(t1[:], pvr[:], bvr,
                                                       pc_i(i), OP.add, OP.mult)
                        t2 = abp.tile([128, TC], F32, tag="rt", bufs=4, name="rt2")
                        nc.vector.scalar_tensor_tensor(t2[:], pvi[:], bvi,
                                                       psn_i(i), OP.add, OP.mult)
                        av = abp.tile([128, TC], F32, tag="av", bufs=3)
                        nc.gpsimd.tensor_tensor(av[:], t1[:], t2[:], OP.add)
                        t3 = abp.tile([128, TC], F32, tag="rt", bufs=4, name="rt3")
                        nc.vector.scalar_tensor_tensor(t3[:], pvr[:], bvr,
                                                       psn_i(i), OP.add, OP.mult)
                        t4 = abp.tile([128, TC], F32, tag="rt", bufs=4, name="rt4")
                        nc.vector.scalar_tensor_tensor(t4[:], pvi[:], bvi,
                                                       pc_i(i), OP.add, OP.mult)
                        bv = abp.tile([128, TC], F32, tag="bv", bufs=3)
                        nc.gpsimd.tensor_tensor(bv[:], t3[:], t4[:], OP.subtract)
                        Av = ABp.tile([128, TC], BF16, tag="Av")
                        nc.vector.tensor_tensor_scan(Av[:], nubs[i][:], av[:],
                                                     0.0, OP.mult, OP.add)
                        Bv = ABp.tile([128, TC], BF16, tag="Bv")
                        nc.vector.tensor_tensor_scan(Bv[:], nubs[i][:], bv[:],
                                                     0.0, OP.mult, OP.add)
                        AB[(i, b)] = (Av, Bv)
                        eb = endAB[b]
                        nc.vector.tensor_copy(eb[:, i:i + 1], Av[:, TC - 1:TC])
                        nc.vector.tensor_copy(eb[:, CT + i:CT + i + 1],
                                              Bv[:, TC - 1:TC])

                    def do_ln1(b):
                        return layer_norm(
                            [x[i][:, BS[b]] for i in range(CT)],
                            lambda i: vt["ln1g"][:, co + i:co + i + 1],
                            lambda i: vt["ln1b"][:, co + i:co + i + 1],
                            rhs, BF16, "lnout")

                    def do_in(b, ln1rhs, inw_tiles):
                        endAB[b] = scl.tile([128, 2 * CT], F32, tag="endAB",
                                            bufs=4, name="endAB")

                        for g in range(CT):
                            for kind in range(2):
                                psl = [ps_mm.tile([128, TC], F32, tag="mmps",
                                                  name="mmps")
                                       for _ in range(2)]
                                for ki in range(CT):
                                    for mi in range(2):
                                        col = (2 * g + mi) * 128
                                        nc.tensor.matmul(
                                            psl[mi][:],
                                            inw_tiles[2 * ki + kind]
                                            [:, col:col + 128],
                                            ln1rhs[ki][:],
                                            start=(ki == 0), stop=(ki == CT - 1))
                                if kind == 0:
                                    rot_scan(g, b, psl[0], psl[1])
                                else:
                                    for mi, ps in enumerate(psl):
                                        mt = PERM[12 + 2 * g + mi]
                                        s = sop.tile([128, TC], BF16, tag="so")
                                        nc.scalar.activation(
                                            s[:], ps[:], AF.Silu,
                                            bias=vt["inbv"][:, l * 24 + mt:
                                                            l * 24 + mt + 1])
                                        so[(mt - 2 * CT, b)] = s
                        # boundary exchange
                        nc.sync.dma_start(cc_in[l][b][:], endAB[b][:])
                        nc.gpsimd.collective_compute(
                            "AllGather", OP.bypass, replica_groups=ALL8,
                            ins=[cc_in[l][b][:]], outs=[cc_out[l][b][:]],
                        )
                        gat = scl.tile([128, 2 * CT * NCH], F32, tag="gat",
                                       bufs=4, name="gat")
                        nc.sync.dma_start(
                            gat[:].rearrange("p (c j) -> p c j", j=NCH),
                            cc_out[l][b][:].rearrange("(j p) c -> p c j", p=128),
                        )
                        for i in range(CT):
                            for ab in range(2):
                                cwsl = cwt[:, (l * CT + i) * NCH:(l * CT + i + 1) * NCH]
                                junk = scl.tile([128, NCH], F32, tag="inij",
                                                bufs=2, name="inij")
                                ini = scl.tile([128, 1], F32, tag="ini", bufs=26,
                                               name="ini")
                                c0 = (ab * CT + i) * NCH
                                nc.vector.scalar_tensor_tensor(
                                    junk[:], gat[:, c0:c0 + NCH], 1.0, cwsl,
                                    OP.mult, OP.mult, accum_out=ini[:])
                                inis[(i, ab, b)] = ini

                    def do_post_lnr_out(b):
                        outw_tiles = load_w15(d["w_out"], l, range(2 * CT),
                                              0, CT * 128)
                        bsl = BS[b]
                        ys = [None] * (2 * CT)
                        for i in range(CT):
                            if npws[i] is None:
                                npw = tmp4.tile([128, TC], BF16, tag="npw",
                                                bufs=7, name="npw")
                                nc.scalar.activation(
                                    npw[:], iota_t[:], AF.Exp,
                                    scale=vt["lnnu"][:, co + i:co + i + 1])
                                npws[i] = npw
                            Av, Bv = AB[(i, b)]
                            c0 = tmp4.tile([128, TC], F32, tag="Sc", bufs=4,
                                           name="c0")
                            nc.vector.scalar_tensor_tensor(
                                c0[:], npws[i][:], inis[(i, 0, b)][:], Av[:],
                                OP.mult, OP.add)
                            c1 = tmp4.tile([128, TC], F32, tag="Sc", bufs=4,
                                           name="c1")
                            nc.vector.scalar_tensor_tensor(
                                c1[:], npws[i][:], inis[(i, 1, b)][:], Bv[:],
                                OP.mult, OP.add)
                            t1 = abp.tile([128, TC], F32, tag="rt", bufs=4,
                                          name="pt1")
                            nc.vector.tensor_tensor(t1[:], pc_i(i), c0[:], OP.mult)
                            t2 = abp.tile([128, TC], F32, tag="rt", bufs=4,
                                          name="pt2")
                            nc.vector.tensor_tensor(t2[:], psn_i(i), c1[:], OP.mult)
                            hr = abp.tile([128, TC], F32, tag="av", bufs=3,
                                          name="hr")
                            nc.vector.tensor_tensor(hr[:], t1[:], t2[:], OP.add)
                            yv = ypp.tile([128, TC], F32R, tag="ypart")
                            nc.gpsimd.tensor_tensor(yv[:], hr[:], so[(i, b)][:],
                                                    OP.mult)
                            ys[i] = yv
                            t3 = abp.tile([128, TC], F32, tag="rt2", bufs=4,
                                          name="pt3")
                            nc.gpsimd.tensor_tensor(t3[:], psn_i(i), c0[:], OP.mult)
                            t4 = abp.tile([128, TC], F32, tag="rt2", bufs=4,
                                          name="pt4")
                            nc.gpsimd.tensor_tensor(t4[:], pc_i(i), c1[:], OP.mult)
                            hi = abp.tile([128, TC], F32, tag="bv", bufs=3,
                                          name="hi")
                            nc.gpsimd.tensor_tensor(hi[:], t3[:], t4[:],
                                                    OP.subtract)
                            yv2 = ypp.tile([128, TC], F32R, tag="ypart")
                            nc.gpsimd.tensor_tensor(yv2[:], hi[:],
                                                    so[(CT + i, b)][:], OP.mult)
                            ys[CT + i] = yv2
                        yn = layer_norm(
                            [t[:] for t in ys],
                            lambda i: vt["lnrg"][:, co2 + i:co2 + i + 1],
                            lambda i: vt["lnrb"][:, co2 + i:co2 + i + 1],
                            rhs, BF16, "lnout")

                        def out_cb(m0, psl, b=b):
                            for mi, ps in enumerate(psl):
                                mt = m0 + mi
                                if xn[mt] is None:
                                    xn[mt] = xres.tile([128, T], F32R, tag="x",
                                                       name="xn")
                                nc.vector.scalar_tensor_tensor(
                                    xn[mt][:, bsl], ps[:],
                                    vt["outbv"][:, co + mt:co + mt + 1],
                                    x[mt][:, bsl], OP.add, OP.add)

                        proj(2 * CT, CT,
                             lambda ki, mt: outw_tiles[ki][:, mt * 128:(mt + 1) * 128],
                             yn, 2, out_cb)
                        ln2rhs[b] = layer_norm(
                            [xn[i][:, bsl] for i in range(CT)],
                            lambda i: vt["ln2g"][:, co + i:co + i + 1],
                            lambda i: vt["ln2b"][:, co + i:co + i + 1],
                            rhs, BF16, "lnout")

                    # ---- pipelined layer schedule ----
                    ln1r0 = do_ln1(0)
                    inw_tiles = []
                    for kt in range(CT):
                        for hh in range(2):
                            wt = w3k.tile([128, 12 * 128], BF16, tag="w3k",
                                          name="w3k")
                            nc.sync.dma_start(
                                wt[:], d["w_in"][l, kt, :,
                                                 hh * 1536:(hh + 1) * 1536])
                            inw_tiles.append(wt)
                    do_in(0, ln1r0, inw_tiles)
                    if pending_mlp[0] is not None:
                        pending_mlp[0]()
                    ln1r1 = do_ln1(1)
                    do_in(1, ln1r1, inw_tiles)
                    do_post_lnr_out(0)
                    do_mlp(l, 0, xn, ln2rhs[0], x)
                    do_post_lnr_out(1)

                    def mk_pending(l=l, x_prev=xn, ln2r=ln2rhs, xtgt=x):
                        def run():
                            do_mlp(l, 1, x_prev, ln2r[1], xtgt)
                        return run

                    pending_mlp[0] = mk_pending()

                # final LN + gathers, batch-pipelined
                def do_lnf(b):
                    xf = layer_norm(
                        [x[i][:, BS[b]] for i in range(CT)],
                        lambda i: vt["lnfg"][:, i:i + 1],
                        lambda i: vt["lnfb"][:, i:i + 1],
                        rhs, BF16, "lnout")
                    for i in range(CT):
                        nc.sync.dma_start(xf_in[b][i * 128:(i + 1) * 128, :],
                                          xf[i][:])
                    nc.gpsimd.collective_compute(
                        "AllGather", OP.bypass, replica_groups=ALL8,
                        ins=[xf_in[b][:]], outs=[xf_all[b][:]],
                    )

                do_lnf(0)
                pending_mlp[0]()
                pending_mlp[0] = None
                do_lnf(1)

            # ---------------- logits phase ----------------
            lst = contextlib.ExitStack()
            with lst:
                pwp = lst.enter_context(tc.tile_pool(name="pwp", bufs=12))
                xfp = lst.enter_context(tc.tile_pool(name="xfp", bufs=12))
                outp_p = lst.enter_context(tc.tile_pool(name="outpp", bufs=8))

                cnt = 0
                for v0, v1 in ((0, 7), (7, 13)):
                    gw = VOFF[v1 - 1] + VW[v1 - 1] - VOFF[v0]
                    pwtl = []
                    for kt in range(CT):
                        w = pwp.tile([128, 3584], BF16, tag="pw", name="pwg")
                        nc.sync.dma_start(
                            w[:, :gw], d["pwt"][kt, :, VOFF[v0]:VOFF[v0] + gw])
                        pwtl.append(w)
                    for tb in range(NC):
                        xfb = []
                        for kt in range(CT):
                            xt = xfp.tile([128, T], BF16, tag="xfb")
                            for b in range(2):
                                nc.sync.dma_start(
                                    xt[:, BS[b]],
                                    xf_all[b][tb * D + kt * 128:
                                              tb * D + (kt + 1) * 128, :])
                            xfb.append(xt)
                        for mt in range(4):
                            for vn in range(v0, v1):
                                w = VW[vn]
                                coff = VOFF[vn] - VOFF[v0]
                                psw = ps_mm.tile([128, 512], F32, tag="mmps",
                                                 name="mmpsw")
                                for kt in range(CT):
                                    nc.tensor.matmul(
                                        psw[:, :w],
                                        xfb[kt][:, mt * 128:(mt + 1) * 128],
                                        pwtl[kt][:, coff:coff + w],
                                        start=(kt == 0), stop=(kt == CT - 1),
                                    )
                                ot = outp_p.tile([128, 512], BF16, tag="ot")
                                nc.scalar.activation(ot[:, :w], psw[:, :w],
                                                     AF.Copy)
                                cnt += 1
                                r0 = tb * T + mt * 128
                                nc.sync.dma_start(
                                    outp[r0:r0 + 128, VOFF[vn]:VOFF[vn] + w],
                                    ot[:, :w])
    return d


def _host_prep(inputs):
    f32 = np.float32
    bf = ml_dtypes.bfloat16
    tokens = np.asarray(inputs["tokens"]).astype(np.int64)
    emb = np.asarray(inputs["emb"], dtype=f32)
    theta = np.exp(np.asarray(inputs["theta_log"], dtype=np.float64))
    nu = np.exp(-np.exp(np.asarray(inputs["nu_log"], dtype=np.float64)))
    gamma = np.exp(np.asarray(inputs["gamma_log"], dtype=np.float64))

    def vec_tile(a, per_l):
        a = np.asarray(a, dtype=f32)
        if a.ndim == 1:
            a = a[None, :]
        Ln = a.shape[0]
        out = np.zeros((128, per_l * Ln), f32)
        for l in range(Ln):
            out[:, l * per_l:(l + 1) * per_l] = a[l].reshape(per_l, 128).T
        return out

    def mm_tile(w, ktn, perm=None):
        w = np.asarray(w, dtype=f32)
        Ln, K, M = w.shape
        out = w.reshape(Ln, ktn, 128, M)
        if perm is not None:
            mt = M // 128
            out = out.reshape(Ln, ktn, 128, mt, 128)[:, :, :, perm, :]
            out = out.reshape(Ln, ktn, 128, M)
        return np.ascontiguousarray(out).astype(bf)

    # fold gamma into in_proj v columns + bias
    inw = np.array(inputs["inw"], dtype=f32)
    inb = np.array(inputs["inb"], dtype=f32)
    gm = gamma.astype(f32)
    inw[:, :, :D] *= gm[:, None, :]
    inw[:, :, D:2 * D] *= gm[:, None, :]
    inb[:, :D] *= gm
    inb[:, D:2 * D] *= gm

    base = {
        "iotat": np.broadcast_to(np.arange(1, TC + 1, dtype=f32),
                                 (128, TC)).copy(),
        "nuv": vec_tile(nu.astype(f32), CT),
        "lnnu": vec_tile(np.log(nu).astype(f32), CT),
        "ln1g": vec_tile(inputs["ln1_g"], CT),
        "ln1b": vec_tile(inputs["ln1_b"], CT),
        "ln2g": vec_tile(inputs["ln2_g"], CT),
        "ln2b": vec_tile(inputs["ln2_b"], CT),
        "outbv": vec_tile(inputs["outb"], CT),
        "b2v": vec_tile(inputs["b2"], CT),
        "lnrg": vec_tile(inputs["lnr_g"], 2 * CT),
        "lnrb": vec_tile(inputs["lnr_b"], 2 * CT),
        "inbv": vec_tile(inb, 24),
        "b1v": vec_tile(inputs["b1"], 24),
        "lnfg": vec_tile(inputs["lnf_g"], CT),
        "lnfb": vec_tile(inputs["lnf_b"], CT),
        "w_in": mm_tile(inw, CT, perm=PERM),
        "w_out": mm_tile(inputs["outw"], 2 * CT),
        "w_1": mm_tile(inputs["w1"], CT),
        "w_2": mm_tile(inputs["w2"], 24),
    }

    pw = np.asarray(inputs["pw"], dtype=f32)
    tok = tokens.reshape(B, S)

    in_maps = []
    t_loc = np.arange(TC, dtype=np.float64)
    for k in range(NC):
        rows = np.concatenate([tok[0, k * TC:(k + 1) * TC],
                               tok[1, k * TC:(k + 1) * TC]])
        x0t = np.ascontiguousarray(emb[rows].T.astype(f32))
        tg = k * TC + t_loc
        ang = tg[None, None, :] * theta[:, :, None]
        postc = np.cos(ang).astype(bf)
        posts = np.sin(ang).astype(bf)
        cw = np.zeros((L, CT, 128, NCH), f32)
        for j in range(k):
            wj = nu ** (TC * (k - 1 - j))
            cw[:, :, :, j] = wj.reshape(L, CT, 128).astype(f32)
        vs = min(VSH * k, V)
        ve = min(vs + VSH, V)
        pwk = np.zeros((D, VSH), f32)
        pwk[:, :ve - vs] = pw[:, vs:ve]
        pwt = np.ascontiguousarray(pwk.reshape(CT, 128, VSH)).astype(bf)
        mm = dict(base)
        mm.update({"x0t": x0t, "postc": postc, "posts": posts,
                   "cw": cw, "pwt": pwt})
        in_maps.append(mm)
    return in_maps


_CACHE = {}


def _get_nc():
    if "nc" not in _CACHE:
        nc = bacc.Bacc("TRN2", target_bir_lowering=False, debug=False,
                       num_devices=NC)
        _build(nc)
        nc.compile()
        _CACHE["nc"] = nc
    return _CACHE["nc"]


def kernel(**inputs):
    nc = _get_nc()
    in_maps = _host_prep(inputs)
    res = run_bass_kernel_spmd(nc, in_maps, core_ids=list(range(NC)),
                               trace=False)
    pb = np.asarray(inputs["pb"], dtype=np.float32)
    out = np.empty((B, S, V), np.float32)
    for k in range(NC):
        vs = min(VSH * k, V)
        ve = min(vs + VSH, V)
        o = np.asarray(res.results[k]["outp"]).astype(np.float32)
        o4 = o.reshape(NC, 2, TC, VSH)
        for b in range(B):
            out[b, :, vs:ve] = (o4[:, b].reshape(S, VSH)[:, :ve - vs]
                                + pb[vs:ve])
    return out


# revision 7
# speedup vs baseline: 1.0215x; 1.0215x over previous
"""Trainium2 Bass kernel for the LRU LM (nn_LruLM), v3.

Sharding: each core takes chunk k of BOTH batches (8 chunks of 256 per batch).
The two batches are software-pipelined out of phase so each batch's per-layer
boundary-state AllGather + DVE scan window is covered by the other batch's
matmuls (in_proj / deferred MLP). Weights stream as bf16 (stationary), LN
stats run on f32r moving operands (1 cycle/row), the LRU scan stays f32 with
bf16 rotation tables and bf16 scan outputs. The complex scan is decomposed
into 2 real first-order scans (tensor_tensor_scan); cross-chunk states are
corrected with an 8-way per-layer per-batch AllGather of local end-states.
Logits are vocab-sharded (6284/core) from a bf16 AllGather of final
activations, bf16 weights, bf16 output (f32 + pb on host).
"""

import contextlib

import numpy as np
import ml_dtypes

import concourse.bacc as bacc
import concourse.mybir as mybir
import concourse.tile as tile
from concourse.bass_utils import run_bass_kernel_spmd

AF = mybir.ActivationFunctionType
OP = mybir.AluOpType
F32 = mybir.dt.float32
F32R = mybir.dt.float32r
BF16 = mybir.dt.bfloat16

V, D, L, B, S = 50257, 768, 6, 2, 2048
TC = 256                     # tokens per chunk per batch
T = 2 * TC                   # tokens per core (b0 cols | b1 cols)
NC = 8
NCH = 8                      # chunks per batch
CT = D // 128                # 6 channel tiles
VSH = 6284                   # vocab shard width (12*512 + 140)
VW = [512] * 12 + [140]
VOFF = [sum(VW[:i]) for i in range(13)]
EPS = 1e-5
ALL8 = [list(range(NC))]
# in_proj column order: (vr_i, vi_i) pairs first so each pair's rotation +
# scan starts as soon as its two psums land; o tiles afterwards.
PERM = [0, 6, 1, 7, 2, 8, 3, 9, 4, 10, 5, 11] + list(range(12, 24))


def _build(nc):
    d = {}
    d["x0t"] = nc.dram_tensor("x0t", [D, T], F32R, kind="ExternalInput")
    d["postc"] = nc.dram_tensor("postc", [L, D, TC], BF16, kind="ExternalInput")
    d["posts"] = nc.dram_tensor("posts", [L, D, TC], BF16, kind="ExternalInput")
    d["iotat"] = nc.dram_tensor("iotat", [128, TC], F32, kind="ExternalInput")
    d["cw"] = nc.dram_tensor("cw", [L, CT, 128, NCH], F32, kind="ExternalInput")
    for nm in ["nuv", "lnnu", "ln1g", "ln1b", "ln2g", "ln2b", "outbv", "b2v"]:
        d[nm] = nc.dram_tensor(nm, [128, CT * L], F32, kind="ExternalInput")
    for nm in ["lnrg", "lnrb"]:
        d[nm] = nc.dram_tensor(nm, [128, 2 * CT * L], F32, kind="ExternalInput")
    for nm in ["inbv", "b1v"]:
        d[nm] = nc.dram_tensor(nm, [128, 24 * L], F32, kind="ExternalInput")
    for nm in ["lnfg", "lnfb"]:
        d[nm] = nc.dram_tensor(nm, [128, CT], F32, kind="ExternalInput")
    d["w_in"] = nc.dram_tensor("w_in", [L, CT, 128, 24 * 128], BF16, kind="ExternalInput")
    d["w_out"] = nc.dram_tensor("w_out", [L, 2 * CT, 128, CT * 128], BF16, kind="ExternalInput")
    d["w_1"] = nc.dram_tensor("w_1", [L, CT, 128, 24 * 128], BF16, kind="ExternalInput")
    d["w_2"] = nc.dram_tensor("w_2", [L, 24, 128, CT * 128], BF16, kind="ExternalInput")
    d["pwt"] = nc.dram_tensor("pwt", [CT, 128, VSH], BF16, kind="ExternalInput")
    outp = nc.dram_tensor("outp", [NC * T, VSH], BF16, kind="ExternalOutput")

    cc_in = [[nc.dram_tensor(f"ccin{l}_{b}", [128, 2 * CT], F32) for b in range(2)]
             for l in range(L)]
    cc_out = [[nc.dram_tensor(f"ccout{l}_{b}", [NCH * 128, 2 * CT], F32)
               for b in range(2)] for l in range(L)]
    xf_in = [nc.dram_tensor(f"xfin{b}", [D, TC], BF16) for b in range(2)]
    xf_all = [nc.dram_tensor(f"xfall{b}", [NC * D, TC], BF16, addr_space="Shared")
              for b in range(2)]

    BS = (slice(0, TC), slice(TC, T))

    with tile.TileContext(nc) as tc:
        est = contextlib.ExitStack()
        with est:
            vec = est.enter_context(tc.tile_pool(name="vec", bufs=1))
            rowp = est.enter_context(tc.tile_pool(name="rowp", bufs=6))
            tmp3 = est.enter_context(tc.tile_pool(name="tmp3", bufs=4))
            tmp4 = est.enter_context(tc.tile_pool(name="tmp4", bufs=4))
            ps_sm = est.enter_context(tc.tile_pool(name="pssm", bufs=1, space="PSUM"))
            ps_bc = est.enter_context(tc.tile_pool(name="psbc", bufs=1, space="PSUM"))
            ps_mm = est.enter_context(tc.tile_pool(name="psmm", bufs=6, space="PSUM"))

            ones128f = vec.tile([128, 1], F32, tag="ones128f")
            nc.vector.memset(ones128f[:], 1.0)
            ones128r = vec.tile([128, 1], F32R, tag="ones128r")
            nc.vector.tensor_copy(ones128r[:], ones128f[:])
            onesrow_f = vec.tile([1, 128], F32, tag="onesrowf")
            nc.vector.memset(onesrow_f[:], 1.0)
            onesrow_r = vec.tile([1, 128], F32R, tag="onesrowr")
            nc.vector.tensor_copy(onesrow_r[:], onesrow_f[:])
            epst = vec.tile([1, 1], F32, tag="epst")
            nc.vector.memset(epst[:], EPS)
            iota_t = vec.tile([128, TC], F32, tag="iota")
            nc.sync.dma_start(iota_t[:], d["iotat"][:])
            cwt = vec.tile([128, L * CT * NCH], F32, tag="cwt")
            nc.sync.dma_start(
                cwt[:].rearrange("p (l c j) -> p l c j", l=L, c=CT),
                d["cw"][:].rearrange("l c p j -> p l c j"),
            )

            vt = {}
            for nm in ["nuv", "lnnu", "ln1g", "ln1b", "ln2g", "ln2b", "outbv",
                       "b2v", "lnrg", "lnrb", "inbv", "b1v", "lnfg", "lnfb"]:
                vt[nm] = vec.tile(list(d[nm].shape), F32, tag=nm, name=nm)
                nc.sync.dma_start(vt[nm][:], d[nm][:])

            def layer_norm(xaps, g_ap, b_ap, out_pool, out_dtype, out_tag):
                """LN over channels (partitions, across len(xaps) [128,TC] APs).
                Stats via f32r/bf16 matmuls; returns per-tile rhs tiles."""
                n = len(xaps)
                nch = float(n * 128)
                ps_st = ps_sm.tile([1, 2 * TC], F32, tag="st")
                for i in range(n):
                    nc.tensor.matmul(ps_st[:, 0:TC], ones128r[:], xaps[i],
                                     start=(i == 0), stop=(i == n - 1))
                for i in range(n):
                    sq = tmp3.tile([128, TC], F32R, tag="sqt", bufs=3)
                    nc.scalar.activation(sq[:], xaps[i], AF.Square)
                    nc.tensor.matmul(ps_st[:, TC:2 * TC], ones128r[:], sq[:],
                                     start=(i == 0), stop=(i == n - 1))
                m = rowp.tile([1, TC], F32, tag="lnrow")
                nc.vector.tensor_scalar_mul(m[:], ps_st[:, 0:TC], 1.0 / nch)
                mq = rowp.tile([1, TC], F32, tag="lnrow")
                nc.vector.tensor_scalar_mul(mq[:], ps_st[:, TC:2 * TC], 1.0 / nch)
                nm2 = rowp.tile([1, TC], F32, tag="lnrow")
                nc.vector.scalar_tensor_tensor(nm2[:], m[:], -1.0, m[:],
                                               OP.mult, OP.mult)
                var = rowp.tile([1, TC], F32, tag="lnrow")
                nc.vector.tensor_tensor(var[:], mq[:], nm2[:], OP.add)
                sd = rowp.tile([1, TC], F32, tag="lnrow")
                nc.scalar.activation(sd[:], var[:], AF.Sqrt, bias=epst[:])
                rstd = rowp.tile([1, TC], F32R, tag="lnrowr")
                with nc.allow_low_precision(reason="f32r bits are f32"):
                    nc.vector.reciprocal(rstd[:], sd[:])
                a0 = rowp.tile([1, TC], F32R, tag="lnrowr")
                nc.vector.scalar_tensor_tensor(a0[:], m[:], -1.0, rstd[:],
                                               OP.mult, OP.mult)
                bc = ps_bc.tile([128, 2 * TC], F32, tag="bc")
                nc.tensor.matmul(bc[:, 0:TC], onesrow_r[:], rstd[:],
                                 start=True, stop=True)
                nc.tensor.matmul(bc[:, TC:2 * TC], onesrow_r[:], a0[:],
                                 start=True, stop=True)
                outs = []
                for i in range(n):
                    t1 = tmp3.tile([128, TC], F32, tag="lnt1", bufs=4)
                    nc.vector.tensor_tensor(t1[:], xaps[i], bc[:, 0:TC], OP.mult)
                    t2 = tmp3.tile([128, TC], F32, tag="lnt2", bufs=4)
                    nc.vector.tensor_tensor(t2[:], t1[:], bc[:, TC:2 * TC], OP.add)
                    o = out_pool.tile([128, TC], out_dtype, tag=out_tag)
                    nc.vector.tensor_scalar(o[:], t2[:], g_ap(i), b_ap(i),
                                            OP.mult, OP.add)
                    outs.append(o)
                return outs

            def proj(n_k, n_mt, get_w, rhs_list, grp, group_cb):
                """psum[mi] = sum_k get_w(k, m0+mi).T @ rhs_list[k]."""
                for m0 in range(0, n_mt, grp):
                    g = min(grp, n_mt - m0)
                    psl = [ps_mm.tile([128, TC], F32, tag="mmps", name="mmps")
                           for _ in range(g)]
                    for ki in range(n_k):
                        for mi in range(g):
                            nc.tensor.matmul(psl[mi][:], get_w(ki, m0 + mi),
                                             rhs_list[ki][:],
                                             start=(ki == 0), stop=(ki == n_k - 1))
                    group_cb(m0, psl)

            tst = contextlib.ExitStack()
            with tst:
                xres = tst.enter_context(tc.tile_pool(name="xres", bufs=12))
                rhs = tst.enter_context(tc.tile_pool(name="rhs", bufs=16))
                tabs = tst.enter_context(tc.tile_pool(name="tabs", bufs=3))
                w3k = tst.enter_context(tc.tile_pool(name="w3k", bufs=12))
                w15 = tst.enter_context(tc.tile_pool(name="w15", bufs=14))
                abp = tst.enter_context(tc.tile_pool(name="abp", bufs=10))
                ABp = tst.enter_context(tc.tile_pool(name="ABp", bufs=13))
                sop = tst.enter_context(tc.tile_pool(name="sop", bufs=25))
                ggp = tst.enter_context(tc.tile_pool(name="ggp", bufs=12))
                ypp = tst.enter_context(tc.tile_pool(name="ypp", bufs=12))
                mpp = tst.enter_context(tc.tile_pool(name="mpp", bufs=6))
                scl = tst.enter_context(tc.tile_pool(name="scl", bufs=2))

                x = []
                for i in range(CT):
                    xt = xres.tile([128, T], F32R, tag="x")
                    nc.sync.dma_start(xt[:], d["x0t"][i * 128:(i + 1) * 128, :])
                    x.append(xt)

                pending_mlp = [None]  # deferred b1 MLP of previous layer

                def load_w15(dram, l, kts, c0, c1):
                    tiles = []
                    for kt in kts:
                        wt = w15.tile([128, c1 - c0], BF16, tag="w15", name="w15t")
                        nc.sync.dma_start(wt[:], dram[l, kt, :, c0:c1])
                        tiles.append(wt)
                    return tiles

                def do_mlp(l, b, x_in, ln2rhs, xtgt):
                    """w1 (2 quarter-calls per half) + w2 per hidden half.
                    Writes x_in + mlp in place into xtgt (b's half)."""
                    co = l * CT
                    bsl = BS[b]
                    mparts = {}
                    for p in range(2):
                        gl = [None] * 12
                        for qh in range(2):
                            q = 2 * p + qh
                            w1q = load_w15(d["w_1"], l, range(CT),
                                           q * 768, (q + 1) * 768)

                            def w1_cb(m0, psl, q=q, gl=gl):
                                for mi, ps in enumerate(psl):
                                    mt_abs = q * 6 + m0 + mi
                                    g = ggp.tile([128, TC], BF16, tag="gelu")
                                    nc.scalar.activation(
                                        g[:], ps[:], AF.Gelu,
                                        bias=vt["b1v"][:, l * 24 + mt_abs:
                                                       l * 24 + mt_abs + 1])
                                    gl[qh * 6 + m0 + mi] = g

                            proj(CT, 6,
                                 lambda ki, mt, w1q=w1q: w1q[ki][:, mt * 128:(mt + 1) * 128],
                                 ln2rhs, 2, w1_cb)
                        w2p = load_w15(d["w_2"], l, range(p * 12, p * 12 + 12),
                                       0, CT * 128)

                        def w2_cb(m0, psl, p=p):
                            for mi, ps in enumerate(psl):
                                mt = m0 + mi
                                if p == 0:
                                    pt = mpp.tile([128, TC], BF16, tag="mpart",
                                                  name="mpart")
                                    nc.scalar.activation(pt[:], ps[:], AF.Copy)
                                    mparts[mt] = pt
                                else:
                                    t = tmp3.tile([128, TC], F32, tag="m2t",
                                                  bufs=3, name="m2t")
                                    nc.vector.scalar_tensor_tensor(
                                        t[:], ps[:],
                                        vt["b2v"][:, co + mt:co + mt + 1],
                                        mparts[mt][:], OP.add, OP.add)
                                    nc.vector.tensor_tensor(
                                        xtgt[mt][:, bsl], t[:],
                                        x_in[mt][:, bsl], OP.add)

                        proj(12, 6,
                             lambda ki, mt, w2p=w2p: w2p[ki][:, mt * 128:(mt + 1) * 128],
                             gl, 2, w2_cb)

                for l in range(L):
                    co = l * CT
                    co2 = l * 2 * CT
                    pct = tabs.tile([128, CT * TC], BF16, tag="tab", name="pct")
                    nc.sync.dma_start(
                        pct[:].rearrange("p (c t) -> p c t", c=CT),
                        d["postc"][l].rearrange("(c p) t -> p c t", p=128),
                    )
                    psnt = tabs.tile([128, CT * TC], BF16, tag="tab", name="psnt")
                    nc.sync.dma_start(
                        psnt[:].rearrange("p (c t) -> p c t", c=CT),
                        d["posts"][l].rearrange("(c p) t -> p c t", p=128),
                    )

                    nubs = [None] * CT
                    npws = [None] * CT
                    AB = {}
                    so = {}
                    endAB = {}
                    inis = {}
                    xn = [None] * CT
                    ln2rhs = [None, None]

                    def pc_i(i):
                        return pct[:, i * TC:(i + 1) * TC]

                    def psn_i(i):
                        return psnt[:, i * TC:(i + 1) * TC]

                    def rot_scan(i, b, pvr, pvi, l=l, AB=AB, endAB=endAB,
                                 nubs=nubs):
                        if nubs[i] is None:
                            nub = abp.tile([128, TC], F32, tag="nub", bufs=6,
                                           name="nub")
                            nc.vector.tensor_scalar(
                                nub[:], iota_t[:], 0.0,
                                vt["nuv"][:, co + i:co + i + 1], OP.mult, OP.add)
                            nubs[i] = nub
                        bvr = vt["inbv"][:, l * 24 + i:l * 24 + i + 1]
                        bvi = vt["inbv"][:, l * 24 + CT + i:l * 24 + CT + i + 1]
                        t1 = abp.tile([128, TC], F32, tag="rt", bufs=4, name="rt1")
                        nc.vector.scalar_tensor_tensor(t1[:], pvr[:], bvr,
                                                       pc_i(i), OP.add, OP.mult)
                        t2 = abp.tile([128, TC], F32, tag="rt", bufs=4, name="rt2")
                        nc.vector.scalar_tensor_tensor(t2[:], pvi[:], bvi,
                                                       psn_i(i), OP.add, OP.mult)
                        av = abp.tile([128, TC], F32, tag="av", bufs=3)
                        nc.vector.tensor_tensor(av[:], t1[:], t2[:], OP.add)
                        t3 = abp.tile([128, TC], F32, tag="rt", bufs=4, name="rt3")
                        nc.vector.scalar_tensor_tensor(t3[:], pvr[:], bvr,
                                                       psn_i(i), OP.add, OP.mult)
                        t4 = abp.tile([128, TC], F32, tag="rt", bufs=4, name="rt4")
                        nc.vector.scalar_tensor_tensor(t4[:], pvi[:], bvi,
                                                       pc_i(i), OP.add, OP.mult)
                        bv = abp.tile([128, TC], F32, tag="bv", bufs=3)
                        nc.vector.tensor_tensor(bv[:], t3[:], t4[:], OP.subtract)
                        Av = ABp.tile([128, TC], BF16, tag="Av")
                        nc.vector.tensor_tensor_scan(Av[:], nubs[i][:], av[:],
                                                     0.0, OP.mult, OP.add)
                        Bv = ABp.tile([128, TC], BF16, tag="Bv")
                        nc.vector.tensor_tensor_scan(Bv[:], nubs[i][:], bv[:],
                                                     0.0, OP.mult, OP.add)
                        AB[(i, b)] = (Av, Bv)
                        eb = endAB[b]
                        nc.vector.tensor_copy(eb[:, i:i + 1], Av[:, TC - 1:TC])
                        nc.vector.tensor_copy(eb[:, CT + i:CT + i + 1],
                                              Bv[:, TC - 1:TC])

                    def do_ln1(b):
                        return layer_norm(
                            [x[i][:, BS[b]] for i in range(CT)],
                            lambda i: vt["ln1g"][:, co + i:co + i + 1],
                            lambda i: vt["ln1b"][:, co + i:co + i + 1],
                            rhs, BF16, "lnout")

                    def do_in(b, ln1rhs, inw_tiles):
                        endAB[b] = scl.tile([128, 2 * CT], F32, tag="endAB",
                                            bufs=4, name="endAB")

                        def in_cb(m0, psl, b=b):
                            if m0 < 12:
                                rot_scan(m0 // 2, b, psl[0], psl[1])
                            else:
                                for mi, ps in enumerate(psl):
                                    mt = PERM[m0 + mi]
                                    s = sop.tile([128, TC], BF16, tag="so")
                                    nc.scalar.activation(
                                        s[:], ps[:], AF.Silu,
                                        bias=vt["inbv"][:, l * 24 + mt:
                                                        l * 24 + mt + 1])
                                    so[(mt - 2 * CT, b)] = s

                        proj(CT, 12,
                             lambda ki, mpos: inw_tiles[2 * ki]
                             [:, mpos * 128:(mpos + 1) * 128],
                             ln1rhs, 2, in_cb)
                        # boundary exchange (issued before the o-region mms)
                        nc.sync.dma_start(cc_in[l][b][:], endAB[b][:])
                        nc.gpsimd.collective_compute(
                            "AllGather", OP.bypass, replica_groups=ALL8,
                            ins=[cc_in[l][b][:]], outs=[cc_out[l][b][:]],
                        )
                        proj(CT, 12,
                             lambda ki, mpos: inw_tiles[2 * ki + 1]
                             [:, mpos * 128:(mpos + 1) * 128],
                             ln1rhs, 2,
                             lambda m0, psl: in_cb(m0 + 12, psl))
                        gat = scl.tile([128, 2 * CT * NCH], F32, tag="gat",
                                       bufs=4, name="gat")
                        nc.sync.dma_start(
                            gat[:].rearrange("p (c j) -> p c j", j=NCH),
                            cc_out[l][b][:].rearrange("(j p) c -> p c j", p=128),
                        )
                        for i in range(CT):
                            for ab in range(2):
                                cwsl = cwt[:, (l * CT + i) * NCH:(l * CT + i + 1) * NCH]
                                junk = scl.tile([128, NCH], F32, tag="inij",
                                                bufs=2, name="inij")
                                ini = scl.tile([128, 1], F32, tag="ini", bufs=26,
                                               name="ini")
                                c0 = (ab * CT + i) * NCH
                                nc.vector.scalar_tensor_tensor(
                                    junk[:], gat[:, c0:c0 + NCH], 1.0, cwsl,
                                    OP.mult, OP.mult, accum_out=ini[:])
                                inis[(i, ab, b)] = ini

                    def do_post_lnr_out(b):
                        outw_tiles = load_w15(d["w_out"], l, range(2 * CT),
                                              0, CT * 128)
                        bsl = BS[b]
                        ys = [None] * (2 * CT)
                        for i in range(CT):
                            if npws[i] is None:
                                npw = tmp4.tile([128, TC], BF16, tag="npw",
                                                bufs=7, name="npw")
                                nc.scalar.activation(
                                    npw[:], iota_t[:], AF.Exp,
                                    scale=vt["lnnu"][:, co + i:co + i + 1])
                                npws[i] = npw
                            Av, Bv = AB[(i, b)]
                            c0 = tmp4.tile([128, TC], F32, tag="Sc", bufs=4,
                                           name="c0")
                            nc.vector.scalar_tensor_tensor(
                                c0[:], npws[i][:], inis[(i, 0, b)][:], Av[:],
                                OP.mult, OP.add)
                            c1 = tmp4.tile([128, TC], F32, tag="Sc", bufs=4,
                                           name="c1")
                            nc.vector.scalar_tensor_tensor(
                                c1[:], npws[i][:], inis[(i, 1, b)][:], Bv[:],
                                OP.mult, OP.add)
                            t1 = abp.tile([128, TC], F32, tag="rt", bufs=4,
                                          name="pt1")
                            nc.vector.tensor_tensor(t1[:], pc_i(i), c0[:], OP.mult)
                            t2 = abp.tile([128, TC], F32, tag="rt", bufs=4,
                                          name="pt2")
                            nc.vector.tensor_tensor(t2[:], psn_i(i), c1[:], OP.mult)
                            hr = abp.tile([128, TC], F32, tag="av", bufs=3,
                                          name="hr")
                            nc.vector.tensor_tensor(hr[:], t1[:], t2[:], OP.add)
                            yv = ypp.tile([128, TC], F32R, tag="ypart")
                            nc.gpsimd.tensor_tensor(yv[:], hr[:], so[(i, b)][:],
                                                    OP.mult)
                            ys[i] = yv
                            t3 = abp.tile([128, TC], F32, tag="rt2", bufs=4,
                                          name="pt3")
                            nc.gpsimd.tensor_tensor(t3[:], psn_i(i), c0[:], OP.mult)
                            t4 = abp.tile([128, TC], F32, tag="rt2", bufs=4,
                                          name="pt4")
                            nc.gpsimd.tensor_tensor(t4[:], pc_i(i), c1[:], OP.mult)
                            hi = abp.tile([128, TC], F32, tag="bv", bufs=3,
                                          name="hi")
                            nc.gpsimd.tensor_tensor(hi[:], t3[:], t4[:],
                                                    OP.subtract)
                            yv2 = ypp.tile([128, TC], F32R, tag="ypart")
                            nc.gpsimd.tensor_tensor(yv2[:], hi[:],
                                                    so[(CT + i, b)][:], OP.mult)
                            ys[CT + i] = yv2
                        yn = layer_norm(
                            [t[:] for t in ys],
                            lambda i: vt["lnrg"][:, co2 + i:co2 + i + 1],
                            lambda i: vt["lnrb"][:, co2 + i:co2 + i + 1],
                            rhs, BF16, "lnout")

                        def out_cb(m0, psl, b=b):
                            for mi, ps in enumerate(psl):
                                mt = m0 + mi
                                if xn[mt] is None:
                                    xn[mt] = xres.tile([128, T], F32R, tag="x",
                                                       name="xn")
                                nc.vector.scalar_tensor_tensor(
                                    xn[mt][:, bsl], ps[:],
                                    vt["outbv"][:, co + mt:co + mt + 1],
                                    x[mt][:, bsl], OP.add, OP.add)

                        proj(2 * CT, CT,
                             lambda ki, mt: outw_tiles[ki][:, mt * 128:(mt + 1) * 128],
                             yn, 2, out_cb)
                        ln2rhs[b] = layer_norm(
                            [xn[i][:, bsl] for i in range(CT)],
                            lambda i: vt["ln2g"][:, co + i:co + i + 1],
                            lambda i: vt["ln2b"][:, co + i:co + i + 1],
                            rhs, BF16, "lnout")

                    # ---- pipelined layer schedule ----
                    ln1r0 = do_ln1(0)
                    inw_tiles = []
                    for kt in range(CT):
                        for hh in range(2):
                            wt = w3k.tile([128, 12 * 128], BF16, tag="w3k",
                                          name="w3k")
                            nc.sync.dma_start(
                                wt[:], d["w_in"][l, kt, :,
                                                 hh * 1536:(hh + 1) * 1536])
                            inw_tiles.append(wt)
                    do_in(0, ln1r0, inw_tiles)
                    if pending_mlp[0] is not None:
                        pending_mlp[0]()
                    ln1r1 = do_ln1(1)
                    do_in(1, ln1r1, inw_tiles)
                    do_post_lnr_out(0)
                    do_mlp(l, 0, xn, ln2rhs[0], x)
                    do_post_lnr_out(1)

                    def mk_pending(l=l, x_prev=xn, ln2r=ln2rhs, xtgt=x):
                        def run():
                            do_mlp(l, 1, x_prev, ln2r[1], xtgt)
                        return run

                    pending_mlp[0] = mk_pending()

                # final LN + gathers, batch-pipelined
                def do_lnf(b):
                    xf = layer_norm(
                        [x[i][:, BS[b]] for i in range(CT)],
                        lambda i: vt["lnfg"][:, i:i + 1],
                        lambda i: vt["lnfb"][:, i:i + 1],
                        rhs, BF16, "lnout")
                    for i in range(CT):
                        nc.sync.dma_start(xf_in[b][i * 128:(i + 1) * 128, :],
                                          xf[i][:])
                    nc.gpsimd.collective_compute(
                        "AllGather", OP.bypass, replica_groups=ALL8,
                        ins=[xf_in[b][:]], outs=[xf_all[b][:]],
                    )

                do_lnf(0)
                pending_mlp[0]()
                pending_mlp[0] = None
                do_lnf(1)

            # ---------------- logits phase ----------------
            lst = contextlib.ExitStack()
            with lst:
                pwp = lst.enter_context(tc.tile_pool(name="pwp", bufs=12))
                xfp = lst.enter_context(tc.tile_pool(name="xfp", bufs=12))
                outp_p = lst.enter_context(tc.tile_pool(name="outpp", bufs=8))

                cnt = 0
                for v0, v1 in ((0, 7), (7, 13)):
                    gw = VOFF[v1 - 1] + VW[v1 - 1] - VOFF[v0]
                    pwtl = []
                    for kt in range(CT):
                        w = pwp.tile([128, 3584], BF16, tag="pw", name="pwg")
                        nc.sync.dma_start(
                            w[:, :gw], d["pwt"][kt, :, VOFF[v0]:VOFF[v0] + gw])
                        pwtl.append(w)
                    for tb in range(NC):
                        xfb = []
                        for kt in range(CT):
                            xt = xfp.tile([128, T], BF16, tag="xfb")
                            for b in range(2):
                                nc.sync.dma_start(
                                    xt[:, BS[b]],
                                    xf_all[b][tb * D + kt * 128:
                                              tb * D + (kt + 1) * 128, :])
                            xfb.append(xt)
                        for mt in range(4):
                            for vn in range(v0, v1):
                                w = VW[vn]
                                coff = VOFF[vn] - VOFF[v0]
                                psw = ps_mm.tile([128, 512], F32, tag="mmps",
                                                 name="mmpsw")
                                for kt in range(CT):
                                    nc.tensor.matmul(
                                        psw[:, :w],
                                        xfb[kt][:, mt * 128:(mt + 1) * 128],
                                        pwtl[kt][:, coff:coff + w],
                                        start=(kt == 0), stop=(kt == CT - 1),
                                    )
                                ot = outp_p.tile([128, 512], BF16, tag="ot")
                                nc.scalar.activation(ot[:, :w], psw[:, :w],
                                                     AF.Copy)
                                cnt += 1
                                r0 = tb * T + mt * 128
                                nc.sync.dma_start(
                                    outp[r0:r0 + 128, VOFF[vn]:VOFF[vn] + w],
                                    ot[:, :w])
    return d


def _host_prep(inputs):
    f32 = np.float32
    bf = ml_dtypes.bfloat16
    tokens = np.asarray(inputs["tokens"]).astype(np.int64)
    emb = np.asarray(inputs["emb"], dtype=f32)
    theta = np.exp(np.asarray(inputs["theta_log"], dtype=np.float64))
    nu = np.exp(-np.exp(np.asarray(inputs["nu_log"], dtype=np.float64)))
    gamma = np.exp(np.asarray(inputs["gamma_log"], dtype=np.float64))

    def vec_tile(a, per_l):
        a = np.asarray(a, dtype=f32)
        if a.ndim == 1:
            a = a[None, :]
        Ln = a.shape[0]
        out = np.zeros((128, per_l * Ln), f32)
        for l in range(Ln):
            out[:, l * per_l:(l + 1) * per_l] = a[l].reshape(per_l, 128).T
        return out

    def mm_tile(w, ktn, perm=None):
        w = np.asarray(w, dtype=f32)
        Ln, K, M = w.shape
        out = w.reshape(Ln, ktn, 128, M)
        if perm is not None:
            mt = M // 128
            out = out.reshape(Ln, ktn, 128, mt, 128)[:, :, :, perm, :]
            out = out.reshape(Ln, ktn, 128, M)
        return np.ascontiguousarray(out).astype(bf)

    # fold gamma into in_proj v columns + bias
    inw = np.array(inputs["inw"], dtype=f32)
    inb = np.array(inputs["inb"], dtype=f32)
    gm = gamma.astype(f32)
    inw[:, :, :D] *= gm[:, None, :]
    inw[:, :, D:2 * D] *= gm[:, None, :]
    inb[:, :D] *= gm
    inb[:, D:2 * D] *= gm

    base = {
        "iotat": np.broadcast_to(np.arange(1, TC + 1, dtype=f32),
                                 (128, TC)).copy(),
        "nuv": vec_tile(nu.astype(f32), CT),
        "lnnu": vec_tile(np.log(nu).astype(f32), CT),
        "ln1g": vec_tile(inputs["ln1_g"], CT),
        "ln1b": vec_tile(inputs["ln1_b"], CT),
        "ln2g": vec_tile(inputs["ln2_g"], CT),
        "ln2b": vec_tile(inputs["ln2_b"], CT),
        "outbv": vec_tile(inputs["outb"], CT),
        "b2v": vec_tile(inputs["b2"], CT),
        "lnrg": vec_tile(inputs["lnr_g"], 2 * CT),
        "lnrb": vec_tile(inputs["lnr_b"], 2 * CT),
        "inbv": vec_tile(inb, 24),
        "b1v": vec_tile(inputs["b1"], 24),
        "lnfg": vec_tile(inputs["lnf_g"], CT),
        "lnfb": vec_tile(inputs["lnf_b"], CT),
        "w_in": mm_tile(inw, CT, perm=PERM),
        "w_out": mm_tile(inputs["outw"], 2 * CT),
        "w_1": mm_tile(inputs["w1"], CT),
        "w_2": mm_tile(inputs["w2"], 24),
    }

    pw = np.asarray(inputs["pw"], dtype=f32)
    tok = tokens.reshape(B, S)

    in_maps = []
    t_loc = np.arange(TC, dtype=np.float64)
    for k in range(NC):
        rows = np.concatenate([tok[0, k * TC:(k + 1) * TC],
                               tok[1, k * TC:(k + 1) * TC]])
        x0t = np.ascontiguousarray(emb[rows].T.astype(f32))
        tg = k * TC + t_loc
        ang = tg[None, None, :] * theta[:, :, None]
        postc = np.cos(ang).astype(bf)
        posts = np.sin(ang).astype(bf)
        cw = np.zeros((L, CT, 128, NCH), f32)
        for j in range(k):
            wj = nu ** (TC * (k - 1 - j))
            cw[:, :, :, j] = wj.reshape(L, CT, 128).astype(f32)
        vs = min(VSH * k, V)
        ve = min(vs + VSH, V)
        pwk = np.zeros((D, VSH), f32)
        pwk[:, :ve - vs] = pw[:, vs:ve]
        pwt = np.ascontiguousarray(pwk.reshape(CT, 128, VSH)).astype(bf)
        mm = dict(base)
        mm.update({"x0t": x0t, "postc": postc, "posts": posts,
                   "cw": cw, "pwt": pwt})
        in_maps.append(mm)
    return in_maps


_CACHE = {}


def _get_nc():
    if "nc" not in _CACHE:
        nc = bacc.Bacc("TRN2", target_bir_lowering=False, debug=False,
                       num_devices=NC)
        _build(nc)
        nc.compile()
        _CACHE["nc"] = nc
    return _CACHE["nc"]


def kernel(**inputs):
    nc = _get_nc()
    in_maps = _host_prep(inputs)
    res = run_bass_kernel_spmd(nc, in_maps, core_ids=list(range(NC)),
                               trace=False)
    pb = np.asarray(inputs["pb"], dtype=np.float32)
    out = np.empty((B, S, V), np.float32)
    for k in range(NC):
        vs = min(VSH * k, V)
        ve = min(vs + VSH, V)
        o = np.asarray(res.results[k]["outp"]).astype(np.float32)
        o4 = o.reshape(NC, 2, TC, VSH)
        for b in range(B):
            out[b, :, vs:ve] = (o4[:, b].reshape(S, VSH)[:, :ve - vs]
                                + pb[vs:ve])
    return out


# revision 9
# speedup vs baseline: 1.0277x; 1.0061x over previous
"""Trainium2 Bass kernel for the LRU LM (nn_LruLM), v3.

Sharding: each core takes chunk k of BOTH batches (8 chunks of 256 per batch).
The two batches are software-pipelined out of phase so each batch's per-layer
boundary-state AllGather + DVE scan window is covered by the other batch's
matmuls (in_proj / deferred MLP). Weights stream as bf16 (stationary), LN
stats run on f32r moving operands (1 cycle/row), the LRU scan stays f32 with
bf16 rotation tables and bf16 scan outputs. The complex scan is decomposed
into 2 real first-order scans (tensor_tensor_scan); cross-chunk states are
corrected with an 8-way per-layer per-batch AllGather of local end-states.
Logits are vocab-sharded (6284/core) from a bf16 AllGather of final
activations, bf16 weights, bf16 output (f32 + pb on host).
"""

import contextlib

import numpy as np
import ml_dtypes

import concourse.bacc as bacc
import concourse.mybir as mybir
import concourse.tile as tile
from concourse.bass_utils import run_bass_kernel_spmd

AF = mybir.ActivationFunctionType
OP = mybir.AluOpType
F32 = mybir.dt.float32
F32R = mybir.dt.float32r
BF16 = mybir.dt.bfloat16

V, D, L, B, S = 50257, 768, 6, 2, 2048
TC = 256                     # tokens per chunk per batch
T = 2 * TC                   # tokens per core (b0 cols | b1 cols)
NC = 8
NCH = 8                      # chunks per batch
CT = D // 128                # 6 channel tiles
VSH = 6284                   # vocab shard width (12*512 + 140)
VW = [512] * 12 + [140]
VOFF = [sum(VW[:i]) for i in range(13)]
EPS = 1e-5
ALL8 = [list(range(NC))]
# in_proj column order: (vr_i, vi_i) pairs first so each pair's rotation +
# scan starts as soon as its two psums land; o tiles afterwards.
PERM = [0, 6, 1, 7, 2, 8, 3, 9, 4, 10, 5, 11] + list(range(12, 24))


def _build(nc):
    d = {}
    d["x0t"] = nc.dram_tensor("x0t", [D, T], F32R, kind="ExternalInput")
    d["postc"] = nc.dram_tensor("postc", [L, D, TC], BF16, kind="ExternalInput")
    d["posts"] = nc.dram_tensor("posts", [L, D, TC], BF16, kind="ExternalInput")
    d["iotat"] = nc.dram_tensor("iotat", [128, TC], F32, kind="ExternalInput")
    d["cw"] = nc.dram_tensor("cw", [L, CT, 128, NCH], F32, kind="ExternalInput")
    for nm in ["nuv", "lnnu", "ln1g", "ln1b", "ln2g", "ln2b", "outbv", "b2v"]:
        d[nm] = nc.dram_tensor(nm, [128, CT * L], F32, kind="ExternalInput")
    for nm in ["lnrg", "lnrb"]:
        d[nm] = nc.dram_tensor(nm, [128, 2 * CT * L], F32, kind="ExternalInput")
    for nm in ["inbv", "b1v"]:
        d[nm] = nc.dram_tensor(nm, [128, 24 * L], F32, kind="ExternalInput")
    for nm in ["lnfg", "lnfb"]:
        d[nm] = nc.dram_tensor(nm, [128, CT], F32, kind="ExternalInput")
    d["w_in"] = nc.dram_tensor("w_in", [L, CT, 128, 24 * 128], BF16, kind="ExternalInput")
    d["w_out"] = nc.dram_tensor("w_out", [L, 2 * CT, 128, CT * 128], BF16, kind="ExternalInput")
    d["w_1"] = nc.dram_tensor("w_1", [L, CT, 128, 24 * 128], BF16, kind="ExternalInput")
    d["w_2"] = nc.dram_tensor("w_2", [L, 24, 128, CT * 128], BF16, kind="ExternalInput")
    d["pwt"] = nc.dram_tensor("pwt", [CT, 128, VSH], BF16, kind="ExternalInput")
    outp = nc.dram_tensor("outp", [NC * T, VSH], BF16, kind="ExternalOutput")

    cc_in = [[nc.dram_tensor(f"ccin{l}_{b}", [128, 2 * CT], F32) for b in range(2)]
             for l in range(L)]
    cc_out = [[nc.dram_tensor(f"ccout{l}_{b}", [NCH * 128, 2 * CT], F32)
               for b in range(2)] for l in range(L)]
    xf_in = [nc.dram_tensor(f"xfin{b}", [D, TC], BF16) for b in range(2)]
    xf_all = [nc.dram_tensor(f"xfall{b}", [NC * D, TC], BF16, addr_space="Shared")
              for b in range(2)]

    BS = (slice(0, TC), slice(TC, T))

    with tile.TileContext(nc) as tc:
        est = contextlib.ExitStack()
        with est:
            vec = est.enter_context(tc.tile_pool(name="vec", bufs=1))
            rowp = est.enter_context(tc.tile_pool(name="rowp", bufs=6))
            tmp3 = est.enter_context(tc.tile_pool(name="tmp3", bufs=4))
            tmp4 = est.enter_context(tc.tile_pool(name="tmp4", bufs=4))
            ps_sm = est.enter_context(tc.tile_pool(name="pssm", bufs=1, space="PSUM"))
            ps_bc = est.enter_context(tc.tile_pool(name="psbc", bufs=1, space="PSUM"))
            ps_mm = est.enter_context(tc.tile_pool(name="psmm", bufs=6, space="PSUM"))

            ones128f = vec.tile([128, 1], F32, tag="ones128f")
            nc.vector.memset(ones128f[:], 1.0)
            ones128r = vec.tile([128, 1], F32R, tag="ones128r")
            nc.vector.tensor_copy(ones128r[:], ones128f[:])
            onesrow_f = vec.tile([1, 128], F32, tag="onesrowf")
            nc.vector.memset(onesrow_f[:], 1.0)
            onesrow_r = vec.tile([1, 128], F32R, tag="onesrowr")
            nc.vector.tensor_copy(onesrow_r[:], onesrow_f[:])
            epst = vec.tile([1, 1], F32, tag="epst")
            nc.vector.memset(epst[:], EPS)
            iota_t = vec.tile([128, TC], F32, tag="iota")
            nc.sync.dma_start(iota_t[:], d["iotat"][:])
            cwt = vec.tile([128, L * CT * NCH], F32, tag="cwt")
            nc.sync.dma_start(
                cwt[:].rearrange("p (l c j) -> p l c j", l=L, c=CT),
                d["cw"][:].rearrange("l c p j -> p l c j"),
            )

            vt = {}
            for nm in ["nuv", "lnnu", "ln1g", "ln1b", "ln2g", "ln2b", "outbv",
                       "b2v", "lnrg", "lnrb", "inbv", "b1v", "lnfg", "lnfb"]:
                vt[nm] = vec.tile(list(d[nm].shape), F32, tag=nm, name=nm)
                nc.sync.dma_start(vt[nm][:], d[nm][:])

            def layer_norm(xaps, g_ap, b_ap, out_pool, out_dtype, out_tag):
                """LN over channels (partitions, across len(xaps) [128,TC] APs).
                Stats via f32r/bf16 matmuls; returns per-tile rhs tiles."""
                n = len(xaps)
                nch = float(n * 128)
                ps_st = ps_sm.tile([1, 2 * TC], F32, tag="st")
                for i in range(n):
                    nc.tensor.matmul(ps_st[:, 0:TC], ones128r[:], xaps[i],
                                     start=(i == 0), stop=(i == n - 1))
                for i in range(n):
                    sq = tmp3.tile([128, TC], F32R, tag="sqt", bufs=3)
                    nc.scalar.activation(sq[:], xaps[i], AF.Square)
                    nc.tensor.matmul(ps_st[:, TC:2 * TC], ones128r[:], sq[:],
                                     start=(i == 0), stop=(i == n - 1))
                m = rowp.tile([1, TC], F32, tag="lnrow")
                nc.vector.tensor_scalar_mul(m[:], ps_st[:, 0:TC], 1.0 / nch)
                mq = rowp.tile([1, TC], F32, tag="lnrow")
                nc.vector.tensor_scalar_mul(mq[:], ps_st[:, TC:2 * TC], 1.0 / nch)
                nm2 = rowp.tile([1, TC], F32, tag="lnrow")
                nc.vector.scalar_tensor_tensor(nm2[:], m[:], -1.0, m[:],
                                               OP.mult, OP.mult)
                var = rowp.tile([1, TC], F32, tag="lnrow")
                nc.vector.tensor_tensor(var[:], mq[:], nm2[:], OP.add)
                sd = rowp.tile([1, TC], F32, tag="lnrow")
                nc.scalar.activation(sd[:], var[:], AF.Sqrt, bias=epst[:])
                rstd = rowp.tile([1, TC], F32R, tag="lnrowr")
                with nc.allow_low_precision(reason="f32r bits are f32"):
                    nc.vector.reciprocal(rstd[:], sd[:])
                a0 = rowp.tile([1, TC], F32R, tag="lnrowr")
                nc.vector.scalar_tensor_tensor(a0[:], m[:], -1.0, rstd[:],
                                               OP.mult, OP.mult)
                bc = ps_bc.tile([128, 2 * TC], F32, tag="bc")
                nc.tensor.matmul(bc[:, 0:TC], onesrow_r[:], rstd[:],
                                 start=True, stop=True)
                nc.tensor.matmul(bc[:, TC:2 * TC], onesrow_r[:], a0[:],
                                 start=True, stop=True)
                outs = []
                for i in range(n):
                    t1 = tmp3.tile([128, TC], F32, tag="lnt1", bufs=4)
                    nc.vector.tensor_tensor(t1[:], xaps[i], bc[:, 0:TC], OP.mult)
                    t2 = tmp3.tile([128, TC], F32, tag="lnt2", bufs=4)
                    nc.vector.tensor_tensor(t2[:], t1[:], bc[:, TC:2 * TC], OP.add)
                    o = out_pool.tile([128, TC], out_dtype, tag=out_tag)
                    nc.vector.tensor_scalar(o[:], t2[:], g_ap(i), b_ap(i),
                                            OP.mult, OP.add)
                    outs.append(o)
                return outs

            def proj(n_k, n_mt, get_w, rhs_list, grp, group_cb):
                """psum[mi] = sum_k get_w(k, m0+mi).T @ rhs_list[k]."""
                for m0 in range(0, n_mt, grp):
                    g = min(grp, n_mt - m0)
                    psl = [ps_mm.tile([128, TC], F32, tag="mmps", name="mmps")
                           for _ in range(g)]
                    for ki in range(n_k):
                        for mi in range(g):
                            nc.tensor.matmul(psl[mi][:], get_w(ki, m0 + mi),
                                             rhs_list[ki][:],
                                             start=(ki == 0), stop=(ki == n_k - 1))
                    group_cb(m0, psl)

            tst = contextlib.ExitStack()
            with tst:
                xres = tst.enter_context(tc.tile_pool(name="xres", bufs=12))
                rhs = tst.enter_context(tc.tile_pool(name="rhs", bufs=16))
                tabs = tst.enter_context(tc.tile_pool(name="tabs", bufs=3))
                w3k = tst.enter_context(tc.tile_pool(name="w3k", bufs=12))
                w15 = tst.enter_context(tc.tile_pool(name="w15", bufs=14))
                abp = tst.enter_context(tc.tile_pool(name="abp", bufs=10))
                ABp = tst.enter_context(tc.tile_pool(name="ABp", bufs=13))
                sop = tst.enter_context(tc.tile_pool(name="sop", bufs=25))
                ggp = tst.enter_context(tc.tile_pool(name="ggp", bufs=12))
                ypp = tst.enter_context(tc.tile_pool(name="ypp", bufs=12))
                mpp = tst.enter_context(tc.tile_pool(name="mpp", bufs=6))
                scl = tst.enter_context(tc.tile_pool(name="scl", bufs=2))

                x = []
                for i in range(CT):
                    xt = xres.tile([128, T], F32R, tag="x")
                    nc.sync.dma_start(xt[:], d["x0t"][i * 128:(i + 1) * 128, :])
                    x.append(xt)

                pending_mlp = [None]  # deferred b1 MLP of previous layer

                def load_w15(dram, l, kts, c0, c1):
                    tiles = []
                    for kt in kts:
                        wt = w15.tile([128, c1 - c0], BF16, tag="w15", name="w15t")
                        nc.sync.dma_start(wt[:], dram[l, kt, :, c0:c1])
                        tiles.append(wt)
                    return tiles

                def do_mlp(l, b, x_in, ln2rhs, xtgt):
                    """w1 (2 quarter-calls per half) + w2 per hidden half.
                    Writes x_in + mlp in place into xtgt (b's half)."""
                    co = l * CT
                    bsl = BS[b]
                    mparts = {}
                    for p in range(2):
                        gl = [None] * 12
                        for qh in range(2):
                            q = 2 * p + qh
                            w1q = load_w15(d["w_1"], l, range(CT),
                                           q * 768, (q + 1) * 768)

                            def w1_cb(m0, psl, q=q, gl=gl):
                                for mi, ps in enumerate(psl):
                                    mt_abs = q * 6 + m0 + mi
                                    g = ggp.tile([128, TC], BF16, tag="gelu")
                                    nc.scalar.activation(
                                        g[:], ps[:], AF.Gelu,
                                        bias=vt["b1v"][:, l * 24 + mt_abs:
                                                       l * 24 + mt_abs + 1])
                                    gl[qh * 6 + m0 + mi] = g

                            proj(CT, 6,
                                 lambda ki, mt, w1q=w1q: w1q[ki][:, mt * 128:(mt + 1) * 128],
                                 ln2rhs, 3, w1_cb)
                        w2p = load_w15(d["w_2"], l, range(p * 12, p * 12 + 12),
                                       0, CT * 128)

                        def w2_cb(m0, psl, p=p):
                            for mi, ps in enumerate(psl):
                                mt = m0 + mi
                                if p == 0:
                                    pt = mpp.tile([128, TC], BF16, tag="mpart",
                                                  name="mpart")
                                    nc.scalar.activation(pt[:], ps[:], AF.Copy)
                                    mparts[mt] = pt
                                else:
                                    t = tmp3.tile([128, TC], F32, tag="m2t",
                                                  bufs=3, name="m2t")
                                    nc.vector.scalar_tensor_tensor(
                                        t[:], ps[:],
                                        vt["b2v"][:, co + mt:co + mt + 1],
                                        mparts[mt][:], OP.add, OP.add)
                                    nc.vector.tensor_tensor(
                                        xtgt[mt][:, bsl], t[:],
                                        x_in[mt][:, bsl], OP.add)

                        proj(12, 6,
                             lambda ki, mt, w2p=w2p: w2p[ki][:, mt * 128:(mt + 1) * 128],
                             gl, 3, w2_cb)

                for l in range(L):
                    co = l * CT
                    co2 = l * 2 * CT
                    pct = tabs.tile([128, CT * TC], BF16, tag="tab", name="pct")
                    nc.sync.dma_start(
                        pct[:].rearrange("p (c t) -> p c t", c=CT),
                        d["postc"][l].rearrange("(c p) t -> p c t", p=128),
                    )
                    psnt = tabs.tile([128, CT * TC], BF16, tag="tab", name="psnt")
                    nc.sync.dma_start(
                        psnt[:].rearrange("p (c t) -> p c t", c=CT),
                        d["posts"][l].rearrange("(c p) t -> p c t", p=128),
                    )

                    nubs = [None] * CT
                    npws = [None] * CT
                    AB = {}
                    so = {}
                    endAB = {}
                    inis = {}
                    xn = [None] * CT
                    ln2rhs = [None, None]

                    def pc_i(i):
                        return pct[:, i * TC:(i + 1) * TC]

                    def psn_i(i):
                        return psnt[:, i * TC:(i + 1) * TC]

                    def rot_scan(i, b, pvr, pvi, l=l, AB=AB, endAB=endAB,
                                 nubs=nubs):
                        if nubs[i] is None:
                            nub = abp.tile([128, TC], F32, tag="nub", bufs=6,
                                           name="nub")
                            nc.vector.tensor_scalar(
                                nub[:], iota_t[:], 0.0,
                                vt["nuv"][:, co + i:co + i + 1], OP.mult, OP.add)
                            nubs[i] = nub
                        bvr = vt["inbv"][:, l * 24 + i:l * 24 + i + 1]
                        bvi = vt["inbv"][:, l * 24 + CT + i:l * 24 + CT + i + 1]
                        t1 = abp.tile([128, TC], F32, tag="rt", bufs=4, name="rt1")
                        nc.vector.scalar_tensor_tensor(t1[:], pvr[:], bvr,
                                                       pc_i(i), OP.add, OP.mult)
                        t2 = abp.tile([128, TC], F32, tag="rt", bufs=4, name="rt2")
                        nc.vector.scalar_tensor_tensor(t2[:], pvi[:], bvi,
                                                       psn_i(i), OP.add, OP.mult)
                        av = abp.tile([128, TC], F32, tag="av", bufs=3)
                        nc.vector.tensor_tensor(av[:], t1[:], t2[:], OP.add)
                        t3 = abp.tile([128, TC], F32, tag="rt", bufs=4, name="rt3")
                        nc.vector.scalar_tensor_tensor(t3[:], pvr[:], bvr,
                                                       psn_i(i), OP.add, OP.mult)
                        t4 = abp.tile([128, TC], F32, tag="rt", bufs=4, name="rt4")
                        nc.vector.scalar_tensor_tensor(t4[:], pvi[:], bvi,
                                                       pc_i(i), OP.add, OP.mult)
                        bv = abp.tile([128, TC], F32, tag="bv", bufs=3)
                        nc.vector.tensor_tensor(bv[:], t3[:], t4[:], OP.subtract)
                        Av = ABp.tile([128, TC], BF16, tag="Av")
                        nc.vector.tensor_tensor_scan(Av[:], nubs[i][:], av[:],
                                                     0.0, OP.mult, OP.add)
                        Bv = ABp.tile([128, TC], BF16, tag="Bv")
                        nc.vector.tensor_tensor_scan(Bv[:], nubs[i][:], bv[:],
                                                     0.0, OP.mult, OP.add)
                        AB[(i, b)] = (Av, Bv)
                        eb = endAB[b]
                        nc.vector.tensor_copy(eb[:, i:i + 1], Av[:, TC - 1:TC])
                        nc.vector.tensor_copy(eb[:, CT + i:CT + i + 1],
                                              Bv[:, TC - 1:TC])

                    def do_ln1(b):
                        return layer_norm(
                            [x[i][:, BS[b]] for i in range(CT)],
                            lambda i: vt["ln1g"][:, co + i:co + i + 1],
                            lambda i: vt["ln1b"][:, co + i:co + i + 1],
                            rhs, BF16, "lnout")

                    def do_in(b, ln1rhs, inw_tiles):
                        endAB[b] = scl.tile([128, 2 * CT], F32, tag="endAB",
                                            bufs=4, name="endAB")

                        def in_cb(m0, psl, b=b):
                            if m0 < 12:
                                rot_scan(m0 // 2, b, psl[0], psl[1])
                            else:
                                for mi, ps in enumerate(psl):
                                    mt = PERM[m0 + mi]
                                    s = sop.tile([128, TC], BF16, tag="so")
                                    nc.scalar.activation(
                                        s[:], ps[:], AF.Silu,
                                        bias=vt["inbv"][:, l * 24 + mt:
                                                        l * 24 + mt + 1])
                                    so[(mt - 2 * CT, b)] = s

                        proj(CT, 12,
                             lambda ki, mpos: inw_tiles[2 * ki]
                             [:, mpos * 128:(mpos + 1) * 128],
                             ln1rhs, 2, in_cb)
                        # boundary exchange (issued before the o-region mms)
                        nc.sync.dma_start(cc_in[l][b][:], endAB[b][:])
                        nc.gpsimd.collective_compute(
                            "AllGather", OP.bypass, replica_groups=ALL8,
                            ins=[cc_in[l][b][:]], outs=[cc_out[l][b][:]],
                        )
                        proj(CT, 12,
                             lambda ki, mpos: inw_tiles[2 * ki + 1]
                             [:, mpos * 128:(mpos + 1) * 128],
                             ln1rhs, 2,
                             lambda m0, psl: in_cb(m0 + 12, psl))
                    def do_gat_inis(b):
                        gat = scl.tile([128, 2 * CT * NCH], F32, tag="gat",
                                       bufs=4, name="gat")
                        nc.sync.dma_start(
                            gat[:].rearrange("p (c j) -> p c j", j=NCH),
                            cc_out[l][b][:].rearrange("(j p) c -> p c j", p=128),
                        )
                        for i in range(CT):
                            for ab in range(2):
                                cwsl = cwt[:, (l * CT + i) * NCH:(l * CT + i + 1) * NCH]
                                junk = scl.tile([128, NCH], F32, tag="inij",
                                                bufs=2, name="inij")
                                ini = scl.tile([128, 1], F32, tag="ini", bufs=26,
                                               name="ini")
                                c0 = (ab * CT + i) * NCH
                                nc.vector.scalar_tensor_tensor(
                                    junk[:], gat[:, c0:c0 + NCH], 1.0, cwsl,
                                    OP.mult, OP.mult, accum_out=ini[:])
                                inis[(i, ab, b)] = ini

                    def do_post_lnr_out(b):
                        do_gat_inis(b)
                        outw_tiles = load_w15(d["w_out"], l, range(2 * CT),
                                              0, CT * 128)
                        bsl = BS[b]
                        ys = [None] * (2 * CT)
                        for i in range(CT):
                            if npws[i] is None:
                                npw = tmp4.tile([128, TC], BF16, tag="npw",
                                                bufs=7, name="npw")
                                nc.scalar.activation(
                                    npw[:], iota_t[:], AF.Exp,
                                    scale=vt["lnnu"][:, co + i:co + i + 1])
                                npws[i] = npw
                            Av, Bv = AB[(i, b)]
                            c0 = tmp4.tile([128, TC], F32, tag="Sc", bufs=4,
                                           name="c0")
                            nc.vector.scalar_tensor_tensor(
                                c0[:], npws[i][:], inis[(i, 0, b)][:], Av[:],
                                OP.mult, OP.add)
                            c1 = tmp4.tile([128, TC], F32, tag="Sc", bufs=4,
                                           name="c1")
                            nc.vector.scalar_tensor_tensor(
                                c1[:], npws[i][:], inis[(i, 1, b)][:], Bv[:],
                                OP.mult, OP.add)
                            t1 = abp.tile([128, TC], F32, tag="rt", bufs=4,
                                          name="pt1")
                            nc.vector.tensor_tensor(t1[:], pc_i(i), c0[:], OP.mult)
                            t2 = abp.tile([128, TC], F32, tag="rt", bufs=4,
                                          name="pt2")
                            nc.vector.tensor_tensor(t2[:], psn_i(i), c1[:], OP.mult)
                            hr = abp.tile([128, TC], F32, tag="av", bufs=3,
                                          name="hr")
                            nc.vector.tensor_tensor(hr[:], t1[:], t2[:], OP.add)
                            yv = ypp.tile([128, TC], F32R, tag="ypart")
                            nc.gpsimd.tensor_tensor(yv[:], hr[:], so[(i, b)][:],
                                                    OP.mult)
                            ys[i] = yv
                            t3 = abp.tile([128, TC], F32, tag="rt2", bufs=4,
                                          name="pt3")
                            nc.gpsimd.tensor_tensor(t3[:], psn_i(i), c0[:], OP.mult)
                            t4 = abp.tile([128, TC], F32, tag="rt2", bufs=4,
                                          name="pt4")
                            nc.gpsimd.tensor_tensor(t4[:], pc_i(i), c1[:], OP.mult)
                            hi = abp.tile([128, TC], F32, tag="bv", bufs=3,
                                          name="hi")
                            nc.gpsimd.tensor_tensor(hi[:], t3[:], t4[:],
                                                    OP.subtract)
                            yv2 = ypp.tile([128, TC], F32R, tag="ypart")
                            nc.gpsimd.tensor_tensor(yv2[:], hi[:],
                                                    so[(CT + i, b)][:], OP.mult)
                            ys[CT + i] = yv2
                        yn = layer_norm(
                            [t[:] for t in ys],
                            lambda i: vt["lnrg"][:, co2 + i:co2 + i + 1],
                            lambda i: vt["lnrb"][:, co2 + i:co2 + i + 1],
                            rhs, BF16, "lnout")

                        def out_cb(m0, psl, b=b):
                            for mi, ps in enumerate(psl):
                                mt = m0 + mi
                                if xn[mt] is None:
                                    xn[mt] = xres.tile([128, T], F32R, tag="x",
                                                       name="xn")
                                nc.vector.scalar_tensor_tensor(
                                    xn[mt][:, bsl], ps[:],
                                    vt["outbv"][:, co + mt:co + mt + 1],
                                    x[mt][:, bsl], OP.add, OP.add)

                        proj(2 * CT, CT,
                             lambda ki, mt: outw_tiles[ki][:, mt * 128:(mt + 1) * 128],
                             yn, 3, out_cb)
                        ln2rhs[b] = layer_norm(
                            [xn[i][:, bsl] for i in range(CT)],
                            lambda i: vt["ln2g"][:, co + i:co + i + 1],
                            lambda i: vt["ln2b"][:, co + i:co + i + 1],
                            rhs, BF16, "lnout")

                    # ---- pipelined layer schedule ----
                    ln1r0 = do_ln1(0)
                    inw_tiles = []
                    for kt in range(CT):
                        for hh in range(2):
                            wt = w3k.tile([128, 12 * 128], BF16, tag="w3k",
                                          name="w3k")
                            nc.sync.dma_start(
                                wt[:], d["w_in"][l, kt, :,
                                                 hh * 1536:(hh + 1) * 1536])
                            inw_tiles.append(wt)
                    do_in(0, ln1r0, inw_tiles)
                    for i in range(CT):
                        npw = tmp4.tile([128, TC], BF16, tag="npw", bufs=7,
                                        name="npw")
                        nc.scalar.activation(
                            npw[:], iota_t[:], AF.Exp,
                            scale=vt["lnnu"][:, co + i:co + i + 1])
                        npws[i] = npw
                    if pending_mlp[0] is not None:
                        pending_mlp[0]()
                    ln1r1 = do_ln1(1)
                    do_in(1, ln1r1, inw_tiles)
                    do_post_lnr_out(0)
                    do_mlp(l, 0, xn, ln2rhs[0], x)
                    do_post_lnr_out(1)

                    def mk_pending(l=l, x_prev=xn, ln2r=ln2rhs, xtgt=x):
                        def run():
                            do_mlp(l, 1, x_prev, ln2r[1], xtgt)
                        return run

                    pending_mlp[0] = mk_pending()

                # final LN + gathers, batch-pipelined
                def do_lnf(b):
                    xf = layer_norm(
                        [x[i][:, BS[b]] for i in range(CT)],
                        lambda i: vt["lnfg"][:, i:i + 1],
                        lambda i: vt["lnfb"][:, i:i + 1],
                        rhs, BF16, "lnout")
                    for i in range(CT):
                        nc.sync.dma_start(xf_in[b][i * 128:(i + 1) * 128, :],
                                          xf[i][:])
                    nc.gpsimd.collective_compute(
                        "AllGather", OP.bypass, replica_groups=ALL8,
                        ins=[xf_in[b][:]], outs=[xf_all[b][:]],
                    )

                do_lnf(0)
                pending_mlp[0]()
                pending_mlp[0] = None
                do_lnf(1)

            # ---------------- logits phase ----------------
            lst = contextlib.ExitStack()
            with lst:
                pwp = lst.enter_context(tc.tile_pool(name="pwp", bufs=12))
                xfp = lst.enter_context(tc.tile_pool(name="xfp", bufs=12))
                outp_p = lst.enter_context(tc.tile_pool(name="outpp", bufs=8))

                cnt = 0
                for v0, v1 in ((0, 7), (7, 13)):
                    gw = VOFF[v1 - 1] + VW[v1 - 1] - VOFF[v0]
                    pwtl = []
                    for kt in range(CT):
                        w = pwp.tile([128, 3584], BF16, tag="pw", name="pwg")
                        nc.sync.dma_start(
                            w[:, :gw], d["pwt"][kt, :, VOFF[v0]:VOFF[v0] + gw])
                        pwtl.append(w)
                    for tb in range(NC):
                        xfb = []
                        for kt in range(CT):
                            xt = xfp.tile([128, T], BF16, tag="xfb")
                            for b in range(2):
                                nc.sync.dma_start(
                                    xt[:, BS[b]],
                                    xf_all[b][tb * D + kt * 128:
                                              tb * D + (kt + 1) * 128, :])
                            xfb.append(xt)
                        for mt in range(4):
                            for vn in range(v0, v1):
                                w = VW[vn]
                                coff = VOFF[vn] - VOFF[v0]
                                psw = ps_mm.tile([128, 512], F32, tag="mmps",
                                                 name="mmpsw")
                                for kt in range(CT):
                                    nc.tensor.matmul(
                                        psw[:, :w],
                                        xfb[kt][:, mt * 128:(mt + 1) * 128],
                                        pwtl[kt][:, coff:coff + w],
                                        start=(kt == 0), stop=(kt == CT - 1),
                                    )
                                ot = outp_p.tile([128, 512], BF16, tag="ot")
                                nc.scalar.activation(ot[:, :w], psw[:, :w],
                                                     AF.Copy)
                                cnt += 1
                                r0 = tb * T + mt * 128
                                nc.sync.dma_start(
                                    outp[r0:r0 + 128, VOFF[vn]:VOFF[vn] + w],
                                    ot[:, :w])
    return d


def _host_prep(inputs):
    f32 = np.float32
    bf = ml_dtypes.bfloat16
    tokens = np.asarray(inputs["tokens"]).astype(np.int64)
    emb = np.asarray(inputs["emb"], dtype=f32)
    theta = np.exp(np.asarray(inputs["theta_log"], dtype=np.float64))
    nu = np.exp(-np.exp(np.asarray(inputs["nu_log"], dtype=np.float64)))
    gamma = np.exp(np.asarray(inputs["gamma_log"], dtype=np.float64))

    def vec_tile(a, per_l):
        a = np.asarray(a, dtype=f32)
        if a.ndim == 1:
            a = a[None, :]
        Ln = a.shape[0]
        out = np.zeros((128, per_l * Ln), f32)
        for l in range(Ln):
            out[:, l * per_l:(l + 1) * per_l] = a[l].reshape(per_l, 128).T
        return out

    def mm_tile(w, ktn, perm=None):
        w = np.asarray(w, dtype=f32)
        Ln, K, M = w.shape
        out = w.reshape(Ln, ktn, 128, M)
        if perm is not None:
            mt = M // 128
            out = out.reshape(Ln, ktn, 128, mt, 128)[:, :, :, perm, :]
            out = out.reshape(Ln, ktn, 128, M)
        return np.ascontiguousarray(out).astype(bf)

    # fold gamma into in_proj v columns + bias
    inw = np.array(inputs["inw"], dtype=f32)
    inb = np.array(inputs["inb"], dtype=f32)
    gm = gamma.astype(f32)
    inw[:, :, :D] *= gm[:, None, :]
    inw[:, :, D:2 * D] *= gm[:, None, :]
    inb[:, :D] *= gm
    inb[:, D:2 * D] *= gm

    base = {
        "iotat": np.broadcast_to(np.arange(1, TC + 1, dtype=f32),
                                 (128, TC)).copy(),
        "nuv": vec_tile(nu.astype(f32), CT),
        "lnnu": vec_tile(np.log(nu).astype(f32), CT),
        "ln1g": vec_tile(inputs["ln1_g"], CT),
        "ln1b": vec_tile(inputs["ln1_b"], CT),
        "ln2g": vec_tile(inputs["ln2_g"], CT),
        "ln2b": vec_tile(inputs["ln2_b"], CT),
        "outbv": vec_tile(inputs["outb"], CT),
        "b2v": vec_tile(inputs["b2"], CT),
        "lnrg": vec_tile(inputs["lnr_g"], 2 * CT),
        "lnrb": vec_tile(inputs["lnr_b"], 2 * CT),
        "inbv": vec_tile(inb, 24),
        "b1v": vec_tile(inputs["b1"], 24),
        "lnfg": vec_tile(inputs["lnf_g"], CT),
        "lnfb": vec_tile(inputs["lnf_b"], CT),
        "w_in": mm_tile(inw, CT, perm=PERM),
        "w_out": mm_tile(inputs["outw"], 2 * CT),
        "w_1": mm_tile(inputs["w1"], CT),
        "w_2": mm_tile(inputs["w2"], 24),
    }

    pw = np.asarray(inputs["pw"], dtype=f32)
    tok = tokens.reshape(B, S)

    in_maps = []
    t_loc = np.arange(TC, dtype=np.float64)
    for k in range(NC):
        rows = np.concatenate([tok[0, k * TC:(k + 1) * TC],
                               tok[1, k * TC:(k + 1) * TC]])
        x0t = np.ascontiguousarray(emb[rows].T.astype(f32))
        tg = k * TC + t_loc
        ang = tg[None, None, :] * theta[:, :, None]
        postc = np.cos(ang).astype(bf)
        posts = np.sin(ang).astype(bf)
        cw = np.zeros((L, CT, 128, NCH), f32)
        for j in range(k):
            wj = nu ** (TC * (k - 1 - j))
            cw[:, :, :, j] = wj.reshape(L, CT, 128).astype(f32)
        vs = min(VSH * k, V)
        ve = min(vs + VSH, V)
        pwk = np.zeros((D, VSH), f32)
        pwk[:, :ve - vs] = pw[:, vs:ve]
        pwt = np.ascontiguousarray(pwk.reshape(CT, 128, VSH)).astype(bf)
        mm = dict(base)
        mm.update({"x0t": x0t, "postc": postc, "posts": posts,
                   "cw": cw, "pwt": pwt})
        in_maps.append(mm)
    return in_maps


_CACHE = {}


def _get_nc():
    if "nc" not in _CACHE:
        nc = bacc.Bacc("TRN2", target_bir_lowering=False, debug=False,
                       num_devices=NC)
        _build(nc)
        nc.compile()
        _CACHE["nc"] = nc
    return _CACHE["nc"]


def kernel(**inputs):
    nc = _get_nc()
    in_maps = _host_prep(inputs)
    res = run_bass_kernel_spmd(nc, in_maps, core_ids=list(range(NC)),
                               trace=False)
    pb = np.asarray(inputs["pb"], dtype=np.float32)
    out = np.empty((B, S, V), np.float32)
    for k in range(NC):
        vs = min(VSH * k, V)
        ve = min(vs + VSH, V)
        o = np.asarray(res.results[k]["outp"]).astype(np.float32)
        o4 = o.reshape(NC, 2, TC, VSH)
        for b in range(B):
            out[b, :, vs:ve] = (o4[:, b].reshape(S, VSH)[:, :ve - vs]
                                + pb[vs:ve])
    return out
